# revision 1
# baseline (speedup 1.0000x reference)
"""Multi-head self-attention on 8 TRN2 NeuronCores.

Full inputs in, full output out. Sharding: tensor-parallel over heads
(4 heads / core) x data-parallel over batch (cores 0-3 -> batch 0,
cores 4-7 -> batch 1). Each core computes a partial [S, D] output
through its 256-row slice of Wo; the host sums the 4 partials per batch
(row-parallel reduce) and adds bo.

Per-core dataflow (all matmuls bf16 with fp32 PSUM accumulation):
  - X^T (pre-transposed on host, bf16) -> Q^T, K^T via W-stationary MMs
  - V in natural [S, 256] layout via X^T-stationary MMs, stored with a
    fused ones-column per head ([V_h | 1] -> M=65 AV matmuls compute
    attn^T and the softmax denominator in one accumulation chain)
  - scores computed transposed (K @ Q^T) so exp runs on ScalarE directly
    from PSUM with the 1/sqrt(hd) scale fused; no max-subtraction (scores
    are O(5) for this distribution; a constant -4 bias guards the range)
  - normalization (1/denom) via DVE reciprocal + GpSimd partition
    broadcast, folded into the PSUM->SBUF copy of attn^T
  - output projection: attn_concat^T stationary, Wo moving, K=256.
"""

import os
import sys
from contextlib import ExitStack

import numpy as np
import ml_dtypes

sys.path.insert(0, "/opt/trn_rl_repo")

import concourse.bass as bass
import concourse.tile as tile
from concourse import bacc, mybir
from concourse import bass_utils

BF16 = mybir.dt.bfloat16
FP32 = mybir.dt.float32
NP_BF16 = ml_dtypes.bfloat16

D = 1024          # d_model
H = 16            # total heads
HD = 64           # head dim
B = 2             # batch
S_FULL = 2048     # sequence length
N_CORES = 8
HPC = 4           # heads per core
CW = HPC * HD     # 256 per-core qkv columns
KT = D // 128     # 8 contraction tiles


def build_mha_kernel(ctx: ExitStack, tc: "tile.TileContext", out_ap: bass.AP,
                     ins: dict, S: int = S_FULL):
    """Emit the per-core MHA kernel body.

    ins: dict of APs: xt [KT,128,S] bf16, wq/wk/wv [KT,128,CW] bf16,
         wo [2,128,D] bf16, bias [3,2,128] f32.
    out_ap: [S, D] f32 partial output.
    """
    nc = tc.nc
    xt_d, wq_d, wk_d, wv_d = ins["xt"], ins["wq"], ins["wk"], ins["wv"]
    wo_d, bias_d = ins["wo"], ins["bias"]

    ST = S // 128                 # seq tiles
    W_JJ = min(S, 1024)           # sq window per scores psum tile
    NJJ = S // W_JJ               # outer sq windows
    NJ2 = W_JJ // 512             # 512-chunks per window
    SCALE = 1.0 / np.sqrt(HD)
    EXP_BIAS = -4.0               # constant shift; cancels in softmax

    const = ctx.enter_context(tc.tile_pool(name="const", bufs=1))

    # ---- persistent SBUF tensors ----
    xt_sb = const.tile([128, KT * S], BF16, tag="xt", name="xt_sb")
    wq_sb = const.tile([128, KT * CW], BF16, tag="wq", name="wq_sb")
    wk_sb = const.tile([128, KT * CW], BF16, tag="wk", name="wk_sb")
    wv_sb = const.tile([128, KT * CW], BF16, tag="wv", name="wv_sb")
    wo_sb = const.tile([128, 2 * D], BF16, tag="wo", name="wo_sb")
    bias_sb = const.tile([128, 6], FP32, tag="bias", name="bias_sb")
    qt_sb = [const.tile([128, S], BF16, tag=f"qt{i}", name=f"qt_sb{i}") for i in range(2)]
    kt_sb = [const.tile([128, S], BF16, tag=f"kt{i}", name=f"kt_sb{i}") for i in range(2)]
    # V with a ones column per head: [V_h0 |1| V_h1 |1| V_h2 |1| V_h3 |1]
    vaug_sb = [const.tile([128, HPC * 65], BF16, tag=f"v{t}", name=f"vaug_sb{t}") for t in range(ST)]
    atT_sb = [const.tile([128, S], BF16, tag=f"at{i}", name=f"atT_sb{i}") for i in range(2)]

    # ---- input DMAs ----
    # dram [k, p, c] -> sbuf [p, k*C + c]
    def load_packed(sb, dram, C):
        nc.sync.dma_start(
            out=sb[:].rearrange("p (k c) -> p k c", k=KT),
            in_=dram.rearrange("k p c -> p k c"),
        )

    load_packed(xt_sb, xt_d, S)
    load_packed(wq_sb, wq_d, CW)
    load_packed(wk_sb, wk_d, CW)
    load_packed(wv_sb, wv_d, CW)
    nc.sync.dma_start(out=wo_sb[:].rearrange("p (k c) -> p k c", k=2),
                      in_=wo_d.rearrange("k p c -> p k c"))
    nc.sync.dma_start(out=bias_sb[:].rearrange("p (m t) -> p m t", m=3),
                      in_=bias_d.rearrange("m t p -> p m t"))

    # ones columns of vaug
    for t in range(ST):
        nc.vector.memset(
            vaug_sb[t][:].rearrange("p (h c) -> p h c", c=65)[:, :, 64:65], 1.0)

    # per-partition constant bias for the exp activation
    ebias_sb = const.tile([128, 1], FP32, tag="ebias", name="ebias_sb")
    nc.vector.memset(ebias_sb[:], EXP_BIAS)

    # ---- PSUM pools: 2+2+2+2 = 8 banks ----
    sc_psum = ctx.enter_context(tc.tile_pool(name="sc", bufs=2, space="PSUM"))
    av_psum = ctx.enter_context(tc.tile_pool(name="av", bufs=NJ2, space="PSUM"))
    pj_psum = ctx.enter_context(tc.tile_pool(name="pj", bufs=2, space="PSUM"))

    exp_pool = ctx.enter_context(tc.tile_pool(name="expp", bufs=3))
    fin_pool = ctx.enter_context(tc.tile_pool(name="fin", bufs=2))
    ost_pool = ctx.enter_context(tc.tile_pool(name="ost", bufs=3))

    # ---- projection chain emitters ----
    def emit_qk_chain(mi, ct, chunk):
        """One 512-col chunk of Q^T (mi=0) or K^T (mi=1) for head pair ct."""
        w_sb, dst = ((wq_sb, qt_sb), (wk_sb, kt_sb))[mi]
        pt = pj_psum.tile([128, 512], FP32, tag="pj", name="pt")
        for k in range(KT):
            nc.tensor.matmul(
                pt[:],
                lhsT=w_sb[:, k * CW + ct * 128: k * CW + ct * 128 + 128],
                rhs=xt_sb[:, k * S + chunk * 512: k * S + chunk * 512 + 512],
                start=(k == 0), stop=(k == KT - 1))
        nc.vector.tensor_scalar_add(
            out=dst[ct][:, chunk * 512: chunk * 512 + 512],
            in0=pt[:],
            scalar1=bias_sb[:, mi * 2 + ct: mi * 2 + ct + 1])

    def emit_v_chain(t):
        """V natural rows t*128..+128 for all 4 heads."""
        pt = pj_psum.tile([128, 512], FP32, tag="pj", name="pt")
        for k in range(KT):
            nc.tensor.matmul(
                pt[:, :CW],
                lhsT=xt_sb[:, k * S + t * 128: k * S + t * 128 + 128],
                rhs=wv_sb[:, k * CW: (k + 1) * CW],
                start=(k == 0), stop=(k == KT - 1))
        for h in range(HPC):
            nc.vector.tensor_copy(
                out=vaug_sb[t][:, h * 65: h * 65 + 64],
                in_=pt[:, h * HD: h * HD + 64])

    def emit_outproj(st):
        """O_partial rows st*128..+128 = attn_concat^T.T @ Wo_c."""
        for nch in range(D // 512):
            pt = pj_psum.tile([128, 512], FP32, tag="pj", name="pt")
            for k2 in range(2):
                nc.tensor.matmul(
                    pt[:],
                    lhsT=atT_sb[k2][:, st * 128: st * 128 + 128],
                    rhs=wo_sb[:, k2 * D + nch * 512: k2 * D + nch * 512 + 512],
                    start=(k2 == 0), stop=(k2 == 1))
            ot = ost_pool.tile([128, 512], FP32, tag="ost", name="ot")
            nc.vector.tensor_copy(out=ot[:], in_=pt[:])
            nc.sync.dma_start(
                out=out_ap[st * 128: st * 128 + 128, nch * 512: nch * 512 + 512],
                in_=ot[:])

    # ---- PE warmup: ~4us of junk matmuls while input DMAs stream ----
    # (HAM clock-gate needs ~3.4us of sustained PE activity to reach 2.4GHz;
    # the scratch input has no DMA dependency so these start immediately)
    wsrc = const.tile([128, 512], BF16, tag="wsrc", name="wsrc")
    nc.vector.memset(wsrc[:], 0.25)
    wup = pj_psum.tile([128, 512], FP32, tag="pj", name="wup")
    for i in range(60):
        nc.tensor.matmul(wup[:], lhsT=wsrc[:, 0:128], rhs=wsrc[:],
                         start=(i == 0), stop=(i == 59))

    # ---- lead-in: only the chains the first block needs up front ----
    NCH = S // 512
    emit_qk_chain(0, 0, 0)
    if NCH > 1:
        emit_qk_chain(0, 0, 1)
    emit_qk_chain(1, 0, 0)

    # remaining projection chains, spread through the attention blocks so
    # the PE absorbs them in ACT-shadow instead of one serial bubble.
    # (jj,h) -> {t: [(mi, ct, chunk), ...]}
    sched = {}
    if S == S_FULL:
        sched = {
            (0, 0): {1: [(1, 0, 1)], 5: [(1, 0, 2)], 9: [(1, 0, 3)]},
            (0, 1): {0: [(0, 0, 2)], 2: [(0, 0, 3)], 4: [(0, 1, 0)],
                     6: [(0, 1, 1)], 8: [(1, 1, 0)]},
            (0, 2): {1: [(1, 1, 1)], 5: [(1, 1, 2)], 9: [(1, 1, 3)]},
            (0, 3): {0: [(0, 1, 2)], 4: [(0, 1, 3)]},
        }
    else:
        for mi in range(2):
            for ct in range(2):
                for chunk in range(NCH):
                    if (mi, ct, chunk) in ((0, 0, 0), (0, 0, 1), (1, 0, 0)):
                        continue
                    sched.setdefault((0, min(1, HPC - 1)), {}).setdefault(
                        0, []).append((mi, ct, chunk))

    def emit_scores(h, jj, t):
        ht, hr = h // 2, (h % 2) * 64
        sc = sc_psum.tile([128, W_JJ], FP32, tag="sc", name="sct")
        for j2 in range(NJ2):
            nc.tensor.matmul(
                sc[:, j2 * 512: (j2 + 1) * 512],
                lhsT=kt_sb[ht][hr: hr + 64, t * 128: t * 128 + 128],
                rhs=qt_sb[ht][hr: hr + 64,
                              jj * W_JJ + j2 * 512: jj * W_JJ + j2 * 512 + 512],
                start=True, stop=True)
        return sc

    # ---- attention: jj outer so output rows finish early; h inner;
    # scores emitted one step ahead of AV to avoid PE head-of-line block
    pending_outproj = []
    for jj in range(NJJ):
        for h in range(HPC):
            ht, hr = h // 2, (h % 2) * 64
            avs = [av_psum.tile([65, 512], FP32, tag="av", name="av") for _ in range(NJ2)]
            sc = emit_scores(h, jj, 0)
            for t in range(ST):
                if jj == 0 and h == 0:
                    emit_v_chain(t)
                for mi, ct, chunk in sched.get((jj, h), {}).get(t, ()):
                    emit_qk_chain(mi, ct, chunk)
                if jj == NJJ - 1 and t % 4 == 2 and pending_outproj:
                    emit_outproj(pending_outproj.pop(0))
                ex = exp_pool.tile([128, W_JJ], BF16, tag="exp", name="ex")
                nc.scalar.activation(ex[:], sc[:],
                                     mybir.ActivationFunctionType.Exp,
                                     bias=ebias_sb[:], scale=SCALE)
                if t + 1 < ST:
                    sc = emit_scores(h, jj, t + 1)   # ahead of AV(t)
                for j2 in range(NJ2):
                    nc.tensor.matmul(
                        avs[j2][:],
                        lhsT=vaug_sb[t][:, h * 65: h * 65 + 65],
                        rhs=ex[:, j2 * 512: (j2 + 1) * 512],
                        start=(t == 0), stop=(t == ST - 1))
            # finalize: release the AV accumulators quickly (raw copies),
            # then normalize off the critical path
            for j2 in range(NJ2):
                col = jj * W_JJ + j2 * 512
                dst = atT_sb[ht][hr: hr + 64, col: col + 512]
                den = fin_pool.tile([1, 512], FP32, tag="den", name="den")
                nc.vector.tensor_copy(den[:], avs[j2][64:65, :])
                nc.vector.tensor_copy(dst, avs[j2][0:64, :])  # unnormalized
                rec = fin_pool.tile([1, 512], FP32, tag="rec", name="rec")
                scr = fin_pool.tile([1, 512], FP32, tag="scr", name="scr")
                nc.vector.reciprocal_approx_accurate(rec[:], den[:], scr[:])
                bc = fin_pool.tile([128, 512], FP32, tag="bc", name="bc")
                nc.gpsimd.partition_broadcast(bc[:], rec[:])
                nc.vector.tensor_mul(dst, dst, bc[hr: hr + 64, :])
                nc.vector.tensor_scalar_add(
                    out=dst, in0=dst,
                    scalar1=bias_sb[hr: hr + 64, 4 + ht: 5 + ht])
                # all heads done for this (jj, j2) column window -> project
                if h == HPC - 1:
                    sts = range((jj * W_JJ + j2 * 512) // 128,
                                (jj * W_JJ + (j2 + 1) * 512) // 128)
                    if jj < NJJ - 1:
                        pending_outproj.extend(sts)   # run inside later blocks
                    else:
                        for st in sts:
                            emit_outproj(st)


def _build_full(S=S_FULL):
    nc = bacc.Bacc("TRN2", target_bir_lowering=False, debug=False,
                   num_devices=N_CORES)
    xt = nc.dram_tensor("xt", [KT, 128, S], BF16, kind="ExternalInput")
    wq = nc.dram_tensor("wq", [KT, 128, CW], BF16, kind="ExternalInput")
    wk = nc.dram_tensor("wk", [KT, 128, CW], BF16, kind="ExternalInput")
    wv = nc.dram_tensor("wv", [KT, 128, CW], BF16, kind="ExternalInput")
    wo = nc.dram_tensor("wo", [2, 128, D], BF16, kind="ExternalInput")
    bias = nc.dram_tensor("bias", [3, 2, 128], FP32, kind="ExternalInput")
    out = nc.dram_tensor("out", [S, D], FP32, kind="ExternalOutput")
    ins = {k: t.ap() for k, t in
           dict(xt=xt, wq=wq, wk=wk, wv=wv, wo=wo, bias=bias).items()}
    with tile.TileContext(nc) as tc:
        with ExitStack() as ctx:
            build_mha_kernel(ctx, tc, out.ap(), ins, S=S)
    nc.compile()
    return nc


def make_core_inputs(X, Wq, bq, Wk, bk, Wv, bv, Wo, bo, S=S_FULL):
    """Build the 8 per-core input maps (host-side shard + transpose + cast)."""
    in_maps = []
    for c in range(N_CORES):
        b = c // (N_CORES // B)
        cs = (c % (N_CORES // B)) * CW
        xt = np.ascontiguousarray(X[b].T).astype(NP_BF16).reshape(KT, 128, S)
        m = {
            "xt": xt,
            "wq": Wq[:, cs:cs + CW].astype(NP_BF16).reshape(KT, 128, CW),
            "wk": Wk[:, cs:cs + CW].astype(NP_BF16).reshape(KT, 128, CW),
            "wv": Wv[:, cs:cs + CW].astype(NP_BF16).reshape(KT, 128, CW),
            "wo": Wo[cs:cs + CW, :].astype(NP_BF16).reshape(2, 128, D),
            "bias": np.stack([bq[cs:cs + CW].reshape(2, 128),
                              bk[cs:cs + CW].reshape(2, 128),
                              bv[cs:cs + CW].reshape(2, 128)]).astype(np.float32),
        }
        in_maps.append(m)
    return in_maps


_NC_CACHE = {}


def _ensure_ntff_hook():
    """Register the axon NTFF profile hook if the image's antenv lacks it."""
    try:
        from antenv.axon_hooks import get_axon_ntff_profile_hook  # noqa: F401
        return  # already available
    except ImportError:
        pass
    try:
        import types
        import antenv
        from trn_agent_boot.trn_boot import _ntff_profile_via_ctypes
        hook = _ntff_profile_via_ctypes("/opt/axon/libaxon_pjrt.so")
        mod = types.ModuleType("antenv.axon_hooks")
        mod._hook = hook
        mod.get_axon_ntff_profile_hook = lambda: mod._hook
        mod.set_axon_ntff_profile_hook = lambda h: setattr(mod, "_hook", h)
        sys.modules["antenv.axon_hooks"] = mod
        antenv.axon_hooks = mod
    except Exception as e:  # profiling degrades, run still works
        print(f"ntff hook setup failed: {e}", file=sys.stderr)


def run_cores(in_maps, S=S_FULL, trace=False, trace_cores=None):
    if trace:
        _ensure_ntff_hook()
    if S not in _NC_CACHE:
        _NC_CACHE[S] = _build_full(S)
    nc = _NC_CACHE[S]
    return bass_utils.run_bass_kernel_spmd(
        nc, in_maps, core_ids=list(range(N_CORES)),
        trace=trace, trace_cores=trace_cores)


def kernel(X, Wq, bq, Wk, bk, Wv, bv, Wo, bo):
    X = np.asarray(X, dtype=np.float32)
    Wq, Wk, Wv, Wo = (np.asarray(w, dtype=np.float32) for w in (Wq, Wk, Wv, Wo))
    bq, bk, bv, bo = (np.asarray(v, dtype=np.float32) for v in (bq, bk, bv, bo))
    S = X.shape[1]
    in_maps = make_core_inputs(X, Wq, bq, Wk, bk, Wv, bv, Wo, bo, S=S)
    res = run_cores(in_maps, S=S)
    out = np.zeros((B, S, D), dtype=np.float32)
    for c in range(N_CORES):
        out[c // (N_CORES // B)] += res.results[c]["out"]
    out += bo
    return out



# revision 3
# speedup vs baseline: 1.0932x; 1.0932x over previous
"""Multi-head self-attention on 8 TRN2 NeuronCores.

Full inputs in, full output out. Sharding: tensor-parallel over heads
(4 heads / core) x data-parallel over batch (cores 0-3 -> batch 0,
cores 4-7 -> batch 1). Each core computes a partial [S, D] output
through its 256-row slice of Wo; the host sums the 4 partials per batch
(row-parallel reduce) and adds bo.

Per-core dataflow (all matmuls bf16 with fp32 PSUM accumulation):
  - scores computed transposed (K @ Q^T) per HEAD PAIR: the two heads of
    a pair live in SBUF partitions 0-63 / 64-127, so their K=64 matmuls
    run CONCURRENTLY on disjoint PE row-tiles (64x128 tiling mode) into
    the two halves of one [128, 1024] PSUM tile.
  - one exp activation per step covers both heads ([128,1024], PSUM src,
    scale=1/sqrt(hd) and a constant -4 bias fused; no max-subtraction)
  - V stored with a fused ones-column per head ([V_h | 1] -> M=65 AV
    matmuls compute attn^T and the softmax denominator in one chain)
  - attention runs as a flat 128-step pipeline (8 blocks = 2 pairs x 4
    query windows of 512, x 16 key chunks); AV trails scores by 2 steps
    so the Scalar engine (the ~140us floor) never bubbles; projection
    chains (QKV, V, output) are slotted into PE slack with explicit
    deadlines.
  - output projection: attn_concat^T stationary, Wo moving, K=256;
    partials DMA'd out in bf16 and reduced on host in fp32.
"""

import os
import sys
from contextlib import ExitStack

import numpy as np
import ml_dtypes

sys.path.insert(0, "/opt/trn_rl_repo")

import concourse.bass as bass
import concourse.tile as tile
from concourse import bacc, mybir
from concourse import bass_utils

BF16 = mybir.dt.bfloat16
FP32 = mybir.dt.float32
NP_BF16 = ml_dtypes.bfloat16

D = 1024          # d_model
H = 16            # total heads
HD = 64           # head dim
B = 2             # batch
S_FULL = 2048     # sequence length
N_CORES = 8
HPC = 4           # heads per core
CW = HPC * HD     # 256 per-core qkv columns
KT = D // 128     # 8 contraction tiles
W = 512           # query window per block
NW = S_FULL // W  # 4 windows
ST = S_FULL // 128  # 16 key chunks


def build_mha_kernel(ctx: ExitStack, tc: "tile.TileContext", out_ap: bass.AP,
                     ins: dict, S: int = S_FULL):
    """Emit the per-core MHA kernel body.

    ins: dict of APs: xt [KT,128,S] bf16, wq/wk/wv [KT,128,CW] bf16,
         wo [2,128,D] bf16, bias [3,2,128] f32.
    out_ap: [S, D] bf16 partial output.
    """
    assert S == S_FULL
    nc = tc.nc
    xt_d, wq_d, wk_d, wv_d = ins["xt"], ins["wq"], ins["wk"], ins["wv"]
    wo_d, bias_d = ins["wo"], ins["bias"]

    SCALE = 1.0 / np.sqrt(HD)
    EXP_BIAS = -4.0               # constant shift; cancels in softmax

    const = ctx.enter_context(tc.tile_pool(name="const", bufs=1))

    # ---- persistent SBUF tensors ----
    xt_sb = const.tile([128, KT * S], BF16, tag="xt", name="xt_sb")
    wq_sb = const.tile([128, KT * CW], BF16, tag="wq", name="wq_sb")
    wk_sb = const.tile([128, KT * CW], BF16, tag="wk", name="wk_sb")
    wv_sb = const.tile([128, KT * CW], BF16, tag="wv", name="wv_sb")
    wo_sb = const.tile([128, 2 * D], BF16, tag="wo", name="wo_sb")
    bias_sb = const.tile([128, 6], FP32, tag="bias", name="bias_sb")
    qt_sb = [const.tile([128, S], BF16, tag=f"qt{i}", name=f"qt_sb{i}") for i in range(2)]
    kt_sb = [const.tile([128, S], BF16, tag=f"kt{i}", name=f"kt_sb{i}") for i in range(2)]
    # V with a ones column per head: [V_h0 |1| V_h1 |1| V_h2 |1| V_h3 |1]
    vaug_sb = [const.tile([128, HPC * 65], BF16, tag=f"v{t}", name=f"vaug_sb{t}") for t in range(ST)]
    atT_sb = [const.tile([128, S], BF16, tag=f"at{i}", name=f"atT_sb{i}") for i in range(2)]

    # ---- input DMAs, split fine so chains start as pieces land ----
    # priority order: bias, wk, wq, xt quarter 0, wv, xt quarters 1-3, wo
    nc.sync.dma_start(out=bias_sb[:].rearrange("p (m t) -> p m t", m=3),
                      in_=bias_d.rearrange("m t p -> p m t"))

    def load_w(sb, dram):
        for k in range(KT):
            nc.sync.dma_start(
                out=sb[:, k * CW:(k + 1) * CW],
                in_=dram[k:k + 1].rearrange("k p c -> p (k c)"))

    def load_xt_quarter(q):
        qs, qe = q * 512, (q + 1) * 512
        for k in range(KT):
            nc.sync.dma_start(
                out=xt_sb[:, k * S + qs: k * S + qe],
                in_=xt_d[k:k + 1, :, qs:qe].rearrange("k p c -> p (k c)"))

    load_w(wk_sb, wk_d)
    load_w(wq_sb, wq_d)
    load_xt_quarter(0)
    load_w(wv_sb, wv_d)
    for q in range(1, 4):
        load_xt_quarter(q)
    for k2 in range(2):
        nc.sync.dma_start(out=wo_sb[:, k2 * D:(k2 + 1) * D],
                          in_=wo_d[k2:k2 + 1].rearrange("k p c -> p (k c)"))

    # ones columns of vaug
    for t in range(ST):
        nc.vector.memset(
            vaug_sb[t][:].rearrange("p (h c) -> p h c", c=65)[:, :, 64:65], 1.0)

    # per-partition constant bias for the exp activation
    ebias_sb = const.tile([128, 1], FP32, tag="ebias", name="ebias_sb")
    nc.vector.memset(ebias_sb[:], EXP_BIAS)

    # ---- PSUM pools: 4 + 2 + 1 + 1 = 8 banks ----
    # qk chains hold their psum tile across two emission steps, so they get
    # a dedicated single-buffer pool; V/outproj/warmup tiles are transient.
    sc_psum = ctx.enter_context(tc.tile_pool(name="sc", bufs=2, space="PSUM"))
    av_psum = ctx.enter_context(tc.tile_pool(name="av", bufs=2, space="PSUM"))
    pjq_psum = ctx.enter_context(tc.tile_pool(name="pjq", bufs=1, space="PSUM"))
    pjv_psum = ctx.enter_context(tc.tile_pool(name="pjv", bufs=1, space="PSUM"))

    exp_pool = ctx.enter_context(tc.tile_pool(name="expp", bufs=4))
    fin_pool = ctx.enter_context(tc.tile_pool(name="fin", bufs=2))
    ost_pool = ctx.enter_context(tc.tile_pool(name="ost", bufs=3))

    # ---- projection emitters ----
    qk_pending = {}   # (mi, ht, c) -> psum tile across the two halves

    def emit_qk_half(mi, ht, c, half):
        """Half (4 k-tiles) of a Q^T (mi=0) / K^T (mi=1) chain for pair ht,
        columns [c*512, (c+1)*512)."""
        w_sb, dst = ((wq_sb, qt_sb), (wk_sb, kt_sb))[mi]
        key = (mi, ht, c)
        if half == 0:
            qk_pending[key] = pj_psum.tile([128, 512], FP32, tag="pj", name="pjt")
        pt = qk_pending[key]
        for k in range(4 * half, 4 * half + 4):
            nc.tensor.matmul(
                pt[:],
                lhsT=w_sb[:, k * CW + ht * 128: k * CW + ht * 128 + 128],
                rhs=xt_sb[:, k * S + c * 512: k * S + c * 512 + 512],
                start=(k == 0), stop=(k == KT - 1))
        if half == 1:
            del qk_pending[key]
            nc.vector.tensor_scalar_add(
                out=dst[ht][:, c * 512: c * 512 + 512],
                in0=pt[:],
                scalar1=bias_sb[:, mi * 2 + ht: mi * 2 + ht + 1])

    def emit_v_pair(pr, t):
        """V natural rows t*128..+128 for head pair pr (2 heads)."""
        pt = pj_psum.tile([128, 512], FP32, tag="pj", name="pjt")
        for k in range(KT):
            nc.tensor.matmul(
                pt[:, 0:128],
                lhsT=xt_sb[:, k * S + t * 128: k * S + t * 128 + 128],
                rhs=wv_sb[:, k * CW + pr * 128: k * CW + pr * 128 + 128],
                start=(k == 0), stop=(k == KT - 1))
        for hh in range(2):
            h = pr * 2 + hh
            nc.vector.tensor_copy(
                out=vaug_sb[t][:, h * 65: h * 65 + 64],
                in_=pt[:, hh * 64: hh * 64 + 64])

    def emit_outproj_unit(st, nch):
        """O_partial rows st*128..+128, cols nch*512..+512."""
        pt = pj_psum.tile([128, 512], FP32, tag="pj", name="pjt")
        for k2 in range(2):
            nc.tensor.matmul(
                pt[:],
                lhsT=atT_sb[k2][:, st * 128: st * 128 + 128],
                rhs=wo_sb[:, k2 * D + nch * 512: k2 * D + nch * 512 + 512],
                start=(k2 == 0), stop=(k2 == 1))
        ot = ost_pool.tile([128, 512], BF16, tag="ost", name="ot")
        nc.vector.tensor_copy(out=ot[:], in_=pt[:])
        nc.sync.dma_start(
            out=out_ap[st * 128: st * 128 + 128, nch * 512: nch * 512 + 512],
            in_=ot[:])

    # ---- PE warmup: junk matmuls bridging engine start -> first DMA pieces ----
    wsrc = const.tile([128, 512], BF16, tag="wsrc", name="wsrc")
    nc.vector.memset(wsrc[:], 0.25)
    wup = pj_psum.tile([128, 512], FP32, tag="pj", name="wup")
    for i in range(12):
        nc.tensor.matmul(wup[:], lhsT=wsrc[:, 0:128], rhs=wsrc[:],
                         start=(i == 0), stop=(i == 11))

    # ---- lead-in chains: exactly what step 0 needs ----
    emit_qk_half(1, 0, 0, 0)
    emit_qk_half(1, 0, 0, 1)   # K^T pair0 cols 0-511 (key chunks 0-3)
    emit_qk_half(0, 0, 0, 0)
    emit_qk_half(0, 0, 0, 1)   # Q^T pair0 cols 0-511 (window 0)
    emit_v_pair(0, 0)
    emit_v_pair(0, 1)

    # ---- per-step work schedule (chains + outproj slotted into PE slack) ----
    work = {s: [] for s in range(130)}

    def addw(s, fn, *a):
        work[s].append((fn, a))

    # block 0 (pair0, window 0): V pair0 JIT + remaining K chains + Q(0,1)
    for s in range(14):
        addw(s, emit_v_pair, 0, s + 2)
    for ci, (s0, s1) in enumerate(((0, 2), (4, 6), (8, 10))):
        addw(s0, emit_qk_half, 1, 0, ci + 1, 0)
        addw(s1, emit_qk_half, 1, 0, ci + 1, 1)
    addw(12, emit_qk_half, 0, 0, 1, 0)
    addw(14, emit_qk_half, 0, 0, 1, 1)
    # block 1: Q(0,2), K(1,0), K(1,1); V pair1 t=0-7 on odd steps
    addw(16, emit_qk_half, 0, 0, 2, 0)
    addw(18, emit_qk_half, 0, 0, 2, 1)
    addw(20, emit_qk_half, 1, 1, 0, 0)
    addw(22, emit_qk_half, 1, 1, 0, 1)
    addw(24, emit_qk_half, 1, 1, 1, 0)
    addw(26, emit_qk_half, 1, 1, 1, 1)
    for i in range(8):
        addw(17 + 2 * i, emit_v_pair, 1, i)
    # block 2: Q(0,3), K(1,2), K(1,3); V pair1 t=8-15 on odd steps
    addw(32, emit_qk_half, 0, 0, 3, 0)
    addw(34, emit_qk_half, 0, 0, 3, 1)
    addw(36, emit_qk_half, 1, 1, 2, 0)
    addw(38, emit_qk_half, 1, 1, 2, 1)
    addw(40, emit_qk_half, 1, 1, 3, 0)
    addw(42, emit_qk_half, 1, 1, 3, 1)
    for i in range(8):
        addw(33 + 2 * i, emit_v_pair, 1, 8 + i)
    # blocks 3-6: Q chains for pair 1
    addw(48, emit_qk_half, 0, 1, 0, 0)
    addw(50, emit_qk_half, 0, 1, 0, 1)
    addw(64, emit_qk_half, 0, 1, 1, 0)
    addw(66, emit_qk_half, 0, 1, 1, 1)
    addw(80, emit_qk_half, 0, 1, 2, 0)
    addw(82, emit_qk_half, 0, 1, 2, 1)
    addw(96, emit_qk_half, 0, 1, 3, 0)
    addw(98, emit_qk_half, 0, 1, 3, 1)
    # outproj: window w complete after block 4+w (steps 64+16w..79+16w).
    # finalize of block 4+w is emitted at step 81+16w.
    for wdx, base in ((0, 84), (1, 100), (2, 116)):
        for u in range(8):
            addw(base + u, emit_outproj_unit, wdx * 4 + u // 2, u % 2)

    def finalize(ht, hh, jw, av):
        """Normalize head hh of pair ht for query window jw from its AV psum."""
        hr = hh * 64
        dst = atT_sb[ht][hr: hr + 64, jw * W: (jw + 1) * W]
        den = fin_pool.tile([1, W], FP32, tag="den", name="den")
        nc.vector.tensor_copy(den[:], av[64:65, :])
        nc.vector.tensor_copy(dst, av[0:64, :])  # unnormalized
        rec = fin_pool.tile([1, W], FP32, tag="rec", name="rec")
        scr = fin_pool.tile([1, W], FP32, tag="scr", name="scr")
        nc.vector.reciprocal_approx_accurate(rec[:], den[:], scr[:])
        bc = fin_pool.tile([128, W], FP32, tag="bc", name="bc")
        nc.gpsimd.partition_broadcast(bc[:], rec[:])
        nc.vector.tensor_mul(dst, dst, bc[hr: hr + 64, :])
        nc.vector.tensor_scalar_add(
            out=dst, in0=dst,
            scalar1=bias_sb[hr: hr + 64, 4 + ht: 5 + ht])

    # ---- attention: flat 128-step pipeline, AV trailing scores by 2 ----
    pending = {}    # step -> ex tile
    block_av = {}   # block -> (av_h0, av_h1)
    for step in range(130):
        if step < 128:
            b, t = step // 16, step % 16
            ht, jw = b // 4, b % 4
            if t == 0:
                block_av[b] = tuple(
                    av_psum.tile([65, W], FP32, tag="av", name="avt")
                    for _ in range(2))
            sc = sc_psum.tile([128, 2 * W], FP32, tag="sc", name="sct")
            for hh in range(2):
                hr = hh * 64
                nc.tensor.matmul(
                    sc[:, hh * W: (hh + 1) * W],
                    lhsT=kt_sb[ht][hr: hr + 64, t * 128: t * 128 + 128],
                    rhs=qt_sb[ht][hr: hr + 64, jw * W: (jw + 1) * W],
                    start=True, stop=True)
            ex = exp_pool.tile([128, 2 * W], BF16, tag="exp", name="ex")
            nc.scalar.activation(ex[:], sc[:],
                                 mybir.ActivationFunctionType.Exp,
                                 bias=ebias_sb[:], scale=SCALE)
            pending[step] = ex
        for fn, a in work[step]:
            fn(*a)
        ps = step - 2
        if ps >= 0:
            ex = pending.pop(ps)
            b2, t2 = ps // 16, ps % 16
            ht2, jw2 = b2 // 4, b2 % 4
            for hh in range(2):
                h = ht2 * 2 + hh
                nc.tensor.matmul(
                    block_av[b2][hh][:],
                    lhsT=vaug_sb[t2][:, h * 65: h * 65 + 65],
                    rhs=ex[:, hh * W: (hh + 1) * W],
                    start=(t2 == 0), stop=(t2 == ST - 1))
            if t2 == ST - 1:
                for hh in range(2):
                    finalize(ht2, hh, jw2, block_av[b2][hh])
                del block_av[b2]

    # ---- tail: output projection for the last window ----
    for u in range(8):
        emit_outproj_unit(3 * 4 + u // 2, u % 2)


def _build_full(S=S_FULL):
    nc = bacc.Bacc("TRN2", target_bir_lowering=False, debug=False,
                   num_devices=N_CORES)
    xt = nc.dram_tensor("xt", [KT, 128, S], BF16, kind="ExternalInput")
    wq = nc.dram_tensor("wq", [KT, 128, CW], BF16, kind="ExternalInput")
    wk = nc.dram_tensor("wk", [KT, 128, CW], BF16, kind="ExternalInput")
    wv = nc.dram_tensor("wv", [KT, 128, CW], BF16, kind="ExternalInput")
    wo = nc.dram_tensor("wo", [2, 128, D], BF16, kind="ExternalInput")
    bias = nc.dram_tensor("bias", [3, 2, 128], FP32, kind="ExternalInput")
    out = nc.dram_tensor("out", [S, D], BF16, kind="ExternalOutput")
    ins = {k: t.ap() for k, t in
           dict(xt=xt, wq=wq, wk=wk, wv=wv, wo=wo, bias=bias).items()}
    with tile.TileContext(nc) as tc:
        with ExitStack() as ctx:
            build_mha_kernel(ctx, tc, out.ap(), ins, S=S)
    nc.compile()
    return nc


def make_core_inputs(X, Wq, bq, Wk, bk, Wv, bv, Wo, bo, S=S_FULL):
    """Build the 8 per-core input maps (host-side shard + transpose + cast)."""
    in_maps = []
    for c in range(N_CORES):
        b = c // (N_CORES // B)
        cs = (c % (N_CORES // B)) * CW
        xt = np.ascontiguousarray(X[b].T).astype(NP_BF16).reshape(KT, 128, S)
        m = {
            "xt": xt,
            "wq": Wq[:, cs:cs + CW].astype(NP_BF16).reshape(KT, 128, CW),
            "wk": Wk[:, cs:cs + CW].astype(NP_BF16).reshape(KT, 128, CW),
            "wv": Wv[:, cs:cs + CW].astype(NP_BF16).reshape(KT, 128, CW),
            "wo": Wo[cs:cs + CW, :].astype(NP_BF16).reshape(2, 128, D),
            "bias": np.stack([bq[cs:cs + CW].reshape(2, 128),
                              bk[cs:cs + CW].reshape(2, 128),
                              bv[cs:cs + CW].reshape(2, 128)]).astype(np.float32),
        }
        in_maps.append(m)
    return in_maps


_NC_CACHE = {}


def _ensure_ntff_hook():
    """Register the axon NTFF profile hook if the image's antenv lacks it."""
    try:
        from antenv.axon_hooks import get_axon_ntff_profile_hook  # noqa: F401
        return  # already available
    except ImportError:
        pass
    try:
        import types
        import antenv
        from trn_agent_boot.trn_boot import _ntff_profile_via_ctypes
        hook = _ntff_profile_via_ctypes("/opt/axon/libaxon_pjrt.so")
        mod = types.ModuleType("antenv.axon_hooks")
        mod._hook = hook
        mod.get_axon_ntff_profile_hook = lambda: mod._hook
        mod.set_axon_ntff_profile_hook = lambda h: setattr(mod, "_hook", h)
        sys.modules["antenv.axon_hooks"] = mod
        antenv.axon_hooks = mod
    except Exception as e:  # profiling degrades, run still works
        print(f"ntff hook setup failed: {e}", file=sys.stderr)


def run_cores(in_maps, S=S_FULL, trace=False, trace_cores=None):
    if trace:
        _ensure_ntff_hook()
    if S not in _NC_CACHE:
        _NC_CACHE[S] = _build_full(S)
    nc = _NC_CACHE[S]
    return bass_utils.run_bass_kernel_spmd(
        nc, in_maps, core_ids=list(range(N_CORES)),
        trace=trace, trace_cores=trace_cores)


def kernel(X, Wq, bq, Wk, bk, Wv, bv, Wo, bo):
    X = np.asarray(X, dtype=np.float32)
    Wq, Wk, Wv, Wo = (np.asarray(w, dtype=np.float32) for w in (Wq, Wk, Wv, Wo))
    bq, bk, bv, bo = (np.asarray(v, dtype=np.float32) for v in (bq, bk, bv, bo))
    S = X.shape[1]
    in_maps = make_core_inputs(X, Wq, bq, Wk, bk, Wv, bv, Wo, bo, S=S)
    res = run_cores(in_maps, S=S)
    out = np.zeros((B, S, D), dtype=np.float32)
    for c in range(N_CORES):
        out[c // (N_CORES // B)] += np.asarray(res.results[c]["out"],
                                               dtype=np.float32)
    out += bo
    return out


# revision 7
# speedup vs baseline: 1.2157x; 1.1121x over previous
"""Multi-head self-attention on 8 TRN2 NeuronCores.

Full inputs in, full output out. Sharding: tensor-parallel over heads
(4 heads / core) x data-parallel over batch (cores 0-3 -> batch 0,
cores 4-7 -> batch 1). Each core computes a partial [S, D] output
through its 256-row slice of Wo; the host sums the 4 partials per batch
(row-parallel reduce) and adds bo + bv @ Wo (the V bias commutes
through softmax, so it's folded into the host-side bias).

Per-core dataflow (all matmuls bf16 with fp32 PSUM accumulation):
  - scores computed transposed (K @ Q^T) per HEAD PAIR: the two heads of
    a pair live in SBUF partitions 0-63 / 64-127, so their K=64 matmuls
    run CONCURRENTLY on disjoint PE row-tiles (64x128 tiling mode) into
    the two halves of one [128, 1024] PSUM tile.
  - one exp activation per step covers both heads ([128,1024], PSUM src,
    scale=1/sqrt(hd) and a constant -4 bias fused; no max-subtraction)
  - V stored with a fused ones-column per head ([V_h | 1] -> M=65 AV
    matmuls compute attn^T and the softmax denominator in one chain)
  - attention runs as a flat 128-step pipeline (8 blocks = 2 pairs x 4
    query windows of 512, x 16 key chunks); AV trails scores by 2 steps
    so the Scalar engine (the ~136us floor) never bubbles; projection
    chains (QKV, V, output) are slotted into PE slack with explicit
    deadlines.
  - output projection: attn_concat^T stationary, Wo moving, K=256;
    each window staged in SBUF bf16 and shipped with one DMA.
"""

import os
import sys
from contextlib import ExitStack

import numpy as np
import ml_dtypes

sys.path.insert(0, "/opt/trn_rl_repo")

import concourse.bass as bass
import concourse.tile as tile
from concourse import bacc, mybir
from concourse import bass_utils

BF16 = mybir.dt.bfloat16
FP32 = mybir.dt.float32
NP_BF16 = ml_dtypes.bfloat16

D = 1024          # d_model
H = 16            # total heads
HD = 64           # head dim
B = 2             # batch
S_FULL = 2048     # sequence length
N_CORES = 8
HPC = 4           # heads per core
CW = HPC * HD     # 256 per-core qkv columns
KT = D // 128     # 8 contraction tiles
W = 512           # query window per block
NW = S_FULL // W  # 4 windows
ST = S_FULL // 128  # 16 key chunks


def build_mha_kernel(ctx: ExitStack, tc: "tile.TileContext", out_ap: bass.AP,
                     ins: dict, S: int = S_FULL):
    """Emit the per-core MHA kernel body.

    ins: dict of APs: xt [KT,128,S] bf16, wq/wk/wv [KT,128,CW] bf16,
         wo [2,128,D] bf16, bias [2,2,128] f32.
    out_ap: [S, D] bf16 partial output (no biases).
    """
    assert S == S_FULL
    nc = tc.nc
    xt_d, wq_d, wk_d, wv_d = ins["xt"], ins["wq"], ins["wk"], ins["wv"]
    wo_d, bias_d = ins["wo"], ins["bias"]

    SCALE = 1.0 / np.sqrt(HD)
    EXP_BIAS = -4.0               # constant shift; cancels in softmax

    const = ctx.enter_context(tc.tile_pool(name="const", bufs=1))

    # ---- persistent SBUF tensors ----
    xt_sb = const.tile([128, KT * S], BF16, tag="xt", name="xt_sb")
    wq_sb = const.tile([128, KT * CW], BF16, tag="wq", name="wq_sb")
    wk_sb = const.tile([128, KT * CW], BF16, tag="wk", name="wk_sb")
    wv_sb = const.tile([128, KT * CW], BF16, tag="wv", name="wv_sb")
    wo_sb = const.tile([128, 2 * D], BF16, tag="wo", name="wo_sb")
    bias_sb = const.tile([128, 4], FP32, tag="bias", name="bias_sb")
    qt_sb = [const.tile([128, S], BF16, tag=f"qt{i}", name=f"qt_sb{i}") for i in range(2)]
    kt_sb = [const.tile([128, S], BF16, tag=f"kt{i}", name=f"kt_sb{i}") for i in range(2)]
    # V with a ones column per head: [V_h0 |1| V_h1 |1| V_h2 |1| V_h3 |1]
    vaug_sb = [const.tile([128, HPC * 65], BF16, tag=f"v{t}", name=f"vaug_sb{t}") for t in range(ST)]
    atT_sb = [const.tile([128, S], BF16, tag=f"at{i}", name=f"atT_sb{i}") for i in range(2)]

    # ---- input DMAs: few coarse kicks, priority-ordered so the first
    # chains (wk, wq, xt cols 0-511) are satisfied earliest ----
    def load_w(sb, dram):
        nc.sync.dma_start(out=sb[:].rearrange("p (k c) -> p k c", k=KT),
                          in_=dram.rearrange("k p c -> p k c"))

    def load_xt_quarter(q, k0, k1):
        qs, qe = q * 512, (q + 1) * 512
        nc.sync.dma_start(
            out=xt_sb[:].rearrange("p (k c) -> p k c", k=KT)[:, k0:k1, qs:qe],
            in_=xt_d[k0:k1, :, qs:qe].rearrange("k p c -> p k c"))

    nc.sync.dma_start(out=bias_sb[:].rearrange("p (m t) -> p m t", m=2),
                      in_=bias_d.rearrange("m t p -> p m t"))
    load_w(wk_sb, wk_d)
    load_w(wq_sb, wq_d)
    load_xt_quarter(0, 0, 4)
    load_xt_quarter(0, 4, 8)
    load_w(wv_sb, wv_d)
    for q in range(1, 4):
        load_xt_quarter(q, 0, 8)
    nc.sync.dma_start(out=wo_sb[:].rearrange("p (k c) -> p k c", k=2),
                      in_=wo_d.rearrange("k p c -> p k c"))

    # ones columns of vaug
    for t in range(ST):
        nc.vector.memset(
            vaug_sb[t][:].rearrange("p (h c) -> p h c", c=65)[:, :, 64:65], 1.0)

    # per-partition constant bias for the exp activation
    ebias_sb = const.tile([128, 1], FP32, tag="ebias", name="ebias_sb")
    nc.vector.memset(ebias_sb[:], EXP_BIAS)

    # ---- PSUM pools: 4 + 2 + 1 + 1 = 8 banks ----
    # qk chains hold their psum tile across two emission steps, so they get
    # a dedicated single-buffer pool; V/outproj/warmup tiles are transient
    # (outproj ping-pongs between the two single-buffer pools).
    sc_psum = ctx.enter_context(tc.tile_pool(name="sc", bufs=2, space="PSUM"))
    av_psum = ctx.enter_context(tc.tile_pool(name="av", bufs=2, space="PSUM"))
    pjq_psum = ctx.enter_context(tc.tile_pool(name="pjq", bufs=1, space="PSUM"))
    pjv_psum = ctx.enter_context(tc.tile_pool(name="pjv", bufs=1, space="PSUM"))

    exp_pool = ctx.enter_context(tc.tile_pool(name="expp", bufs=4))
    fin_pool = ctx.enter_context(tc.tile_pool(name="fin", bufs=2))
    ostw_pool = ctx.enter_context(tc.tile_pool(name="ostw", bufs=2))

    # ---- projection emitters ----
    qk_pending = {}   # (mi, ht, c) -> psum tile across the two halves

    def emit_qk_half(mi, ht, c, half):
        """Half (4 k-tiles) of a Q^T (mi=0) / K^T (mi=1) chain for pair ht,
        columns [c*512, (c+1)*512)."""
        w_sb, dst = ((wq_sb, qt_sb), (wk_sb, kt_sb))[mi]
        key = (mi, ht, c)
        if half == 0:
            qk_pending[key] = pjq_psum.tile([128, 512], FP32, tag="pjq", name="pjqt")
        pt = qk_pending[key]
        for k in range(4 * half, 4 * half + 4):
            nc.tensor.matmul(
                pt[:],
                lhsT=w_sb[:, k * CW + ht * 128: k * CW + ht * 128 + 128],
                rhs=xt_sb[:, k * S + c * 512: k * S + c * 512 + 512],
                start=(k == 0), stop=(k == KT - 1))
        if half == 1:
            del qk_pending[key]
            nc.vector.tensor_scalar_add(
                out=dst[ht][:, c * 512: c * 512 + 512],
                in0=pt[:],
                scalar1=bias_sb[:, mi * 2 + ht: mi * 2 + ht + 1])

    def emit_v_pair(pr, t, pool=None, tag="pj"):
        """V natural rows t*128..+128 for head pair pr (2 heads)."""
        pt = (pool or pjv_psum).tile([128, 512], FP32, tag=tag, name="pjt")
        for k in range(KT):
            nc.tensor.matmul(
                pt[:, 0:128],
                lhsT=xt_sb[:, k * S + t * 128: k * S + t * 128 + 128],
                rhs=wv_sb[:, k * CW + pr * 128: k * CW + pr * 128 + 128],
                start=(k == 0), stop=(k == KT - 1))
        for hh in range(2):
            h = pr * 2 + hh
            nc.vector.tensor_copy(
                out=vaug_sb[t][:, h * 65: h * 65 + 64],
                in_=pt[:, hh * 64: hh * 64 + 64])

    ow_state = {}   # window -> staging tile

    def emit_outproj_unit(wdx, u):
        """One outproj unit: rows (4*wdx + u//2)*128..+128, cols (u%2)*512."""
        st, nch = wdx * 4 + u // 2, u % 2
        pool = pjv_psum if u % 2 == 0 else pjq_psum
        pt = pool.tile([128, 512], FP32, tag="pj" if u % 2 == 0 else "pjq",
                       name="pjt")
        for k2 in range(2):
            nc.tensor.matmul(
                pt[:],
                lhsT=atT_sb[k2][:, st * 128: st * 128 + 128],
                rhs=wo_sb[:, k2 * D + nch * 512: k2 * D + nch * 512 + 512],
                start=(k2 == 0), stop=(k2 == 1))
        if wdx not in ow_state:
            ow_state[wdx] = ostw_pool.tile([128, 4 * D], BF16, tag="ostw",
                                           name="ostw")
        stg = ow_state[wdx]
        nc.vector.tensor_copy(
            out=stg[:, (st % 4) * D + nch * 512: (st % 4) * D + nch * 512 + 512],
            in_=pt[:])
        if u == 7:   # window complete -> single DMA
            del ow_state[wdx]
            nc.sync.dma_start(
                out=out_ap.rearrange("(st p) d -> p st d", p=128)[:, 4 * wdx: 4 * wdx + 4, :],
                in_=stg[:].rearrange("p (st d) -> p st d", st=4))

    # ---- PE warmup: junk matmuls bridging engine start -> first DMA pieces ----
    wsrc = const.tile([128, 512], BF16, tag="wsrc", name="wsrc")
    nc.vector.memset(wsrc[:], 0.25)
    wup = pjv_psum.tile([128, 512], FP32, tag="pj", name="wup")
    for i in range(12):
        nc.tensor.matmul(wup[:], lhsT=wsrc[:, 0:128], rhs=wsrc[:],
                         start=(i == 0), stop=(i == 11))

    # ---- lead-in chains: exactly what step 0 needs ----
    emit_qk_half(1, 0, 0, 0)
    emit_qk_half(1, 0, 0, 1)   # K^T pair0 cols 0-511 (key chunks 0-3)
    emit_qk_half(0, 0, 0, 0)
    emit_qk_half(0, 0, 0, 1)   # Q^T pair0 cols 0-511 (window 0)
    emit_v_pair(0, 0, pool=pjv_psum)
    emit_v_pair(0, 1, pool=pjq_psum, tag="pjq")

    # ---- per-step work schedule (chains + outproj slotted into PE slack) ----
    work = {s: [] for s in range(130)}

    def addw(s, fn, *a):
        work[s].append((fn, a))

    # block 0 (pair0, window 0): V pair0 JIT + remaining K chains + Q(0,1)
    for s in range(14):
        addw(s, emit_v_pair, 0, s + 2)
    for ci, (s0, s1) in enumerate(((0, 2), (4, 6), (8, 10))):
        addw(s0, emit_qk_half, 1, 0, ci + 1, 0)
        addw(s1, emit_qk_half, 1, 0, ci + 1, 1)
    addw(12, emit_qk_half, 0, 0, 1, 0)
    addw(14, emit_qk_half, 0, 0, 1, 1)
    # blocks 1-2: remaining pair-0 Q chains + pair-1 K chains
    addw(16, emit_qk_half, 0, 0, 2, 0)
    addw(18, emit_qk_half, 0, 0, 2, 1)
    addw(20, emit_qk_half, 1, 1, 0, 0)
    addw(22, emit_qk_half, 1, 1, 0, 1)
    addw(24, emit_qk_half, 1, 1, 1, 0)
    addw(26, emit_qk_half, 1, 1, 1, 1)
    addw(32, emit_qk_half, 0, 0, 3, 0)
    addw(34, emit_qk_half, 0, 0, 3, 1)
    addw(36, emit_qk_half, 1, 1, 2, 0)
    addw(38, emit_qk_half, 1, 1, 2, 1)
    addw(40, emit_qk_half, 1, 1, 3, 0)
    addw(42, emit_qk_half, 1, 1, 3, 1)
    # blocks 3-4 (light): pair-1 V chains on odd steps + pair-1 Q chains
    addw(48, emit_qk_half, 0, 1, 0, 0)
    addw(50, emit_qk_half, 0, 1, 0, 1)
    for i in range(8):
        addw(49 + 2 * i, emit_v_pair, 1, i)
    addw(64, emit_qk_half, 0, 1, 1, 0)
    addw(66, emit_qk_half, 0, 1, 1, 1)
    for i in range(8):
        addw(65 + 2 * i, emit_v_pair, 1, 8 + i)
    # blocks 5-7: remaining pair-1 Q chains + outproj of completed windows.
    # window w complete after block 4+w; its finalize lands at step 81+16w.
    addw(80, emit_qk_half, 0, 1, 2, 0)
    addw(82, emit_qk_half, 0, 1, 2, 1)
    addw(96, emit_qk_half, 0, 1, 3, 0)
    addw(98, emit_qk_half, 0, 1, 3, 1)
    for wdx, base in ((0, 84), (1, 100), (2, 116)):
        for u in range(8):
            addw(base + u, emit_outproj_unit, wdx, u)

    def finalize_pair(ht, jw, avp):
        """Normalize both heads of pair ht for query window jw.

        attn^T_unnorm -> atT (bf16), then scaled in place by the
        broadcast reciprocal of the denominator row. No bias (folded
        into the host-side reduce as bv @ Wo).
        """
        dsts, recs = [], []
        for hh in range(2):
            hr = hh * 64
            dst = atT_sb[ht][hr: hr + 64, jw * W: (jw + 1) * W]
            den = fin_pool.tile([1, W], FP32, tag=f"den{hh}", name="den")
            nc.vector.tensor_copy(den[:], avp[hh][64:65, :])
            nc.vector.tensor_copy(dst, avp[hh][0:64, :])
            rec = fin_pool.tile([1, W], FP32, tag=f"rec{hh}", name="rec")
            nc.vector.reciprocal_approx_fast(out=rec[:], in_=den[:])
            dsts.append(dst)
            recs.append(rec)
        bcs = []
        for hh in range(2):
            bc = fin_pool.tile([128, W], FP32, tag=f"bc{hh}", name="bc")
            nc.gpsimd.partition_broadcast(bc[:], recs[hh][:])
            bcs.append(bc)
        for hh in range(2):
            hr = hh * 64
            nc.vector.tensor_mul(dsts[hh], dsts[hh], bcs[hh][hr: hr + 64, :])

    # ---- attention: flat 128-step pipeline, AV trailing scores by 2 ----
    pending = {}    # step -> ex tile
    block_av = {}   # block -> (av_h0, av_h1)
    for step in range(130):
        if step < 128:
            b, t = step // 16, step % 16
            ht, jw = b // 4, b % 4
            if t == 0:
                block_av[b] = tuple(
                    av_psum.tile([65, W], FP32, tag="av", name="avt")
                    for _ in range(2))
            sc = sc_psum.tile([128, 2 * W], FP32, tag="sc", name="sct")
            for hh in range(2):
                hr = hh * 64
                nc.tensor.matmul(
                    sc[:, hh * W: (hh + 1) * W],
                    lhsT=kt_sb[ht][hr: hr + 64, t * 128: t * 128 + 128],
                    rhs=qt_sb[ht][hr: hr + 64, jw * W: (jw + 1) * W],
                    start=True, stop=True)
            ex = exp_pool.tile([128, 2 * W], BF16, tag="exp", name="ex")
            nc.scalar.activation(ex[:], sc[:],
                                 mybir.ActivationFunctionType.Exp,
                                 bias=ebias_sb[:], scale=SCALE)
            pending[step] = ex
        for fn, a in work[step]:
            fn(*a)
        ps = step - 2
        if ps >= 0:
            ex = pending.pop(ps)
            b2, t2 = ps // 16, ps % 16
            ht2, jw2 = b2 // 4, b2 % 4
            for hh in range(2):
                h = ht2 * 2 + hh
                nc.tensor.matmul(
                    block_av[b2][hh][:],
                    lhsT=vaug_sb[t2][:, h * 65: h * 65 + 65],
                    rhs=ex[:, hh * W: (hh + 1) * W],
                    start=(t2 == 0), stop=(t2 == ST - 1))
            if t2 == ST - 1:
                finalize_pair(ht2, jw2, block_av.pop(b2))

    # ---- tail: output projection for the last window ----
    for u in range(8):
        emit_outproj_unit(3, u)


def _build_full(S=S_FULL):
    nc = bacc.Bacc("TRN2", target_bir_lowering=False, debug=False,
                   num_devices=N_CORES)
    xt = nc.dram_tensor("xt", [KT, 128, S], BF16, kind="ExternalInput")
    wq = nc.dram_tensor("wq", [KT, 128, CW], BF16, kind="ExternalInput")
    wk = nc.dram_tensor("wk", [KT, 128, CW], BF16, kind="ExternalInput")
    wv = nc.dram_tensor("wv", [KT, 128, CW], BF16, kind="ExternalInput")
    wo = nc.dram_tensor("wo", [2, 128, D], BF16, kind="ExternalInput")
    bias = nc.dram_tensor("bias", [2, 2, 128], FP32, kind="ExternalInput")
    out = nc.dram_tensor("out", [S, D], BF16, kind="ExternalOutput")
    ins = {k: t.ap() for k, t in
           dict(xt=xt, wq=wq, wk=wk, wv=wv, wo=wo, bias=bias).items()}
    with tile.TileContext(nc) as tc:
        with ExitStack() as ctx:
            build_mha_kernel(ctx, tc, out.ap(), ins, S=S)
    nc.compile()
    return nc


def make_core_inputs(X, Wq, bq, Wk, bk, Wv, bv, Wo, bo, S=S_FULL):
    """Build the 8 per-core input maps (host-side shard + transpose + cast)."""
    in_maps = []
    for c in range(N_CORES):
        b = c // (N_CORES // B)
        cs = (c % (N_CORES // B)) * CW
        xt = np.ascontiguousarray(X[b].T).astype(NP_BF16).reshape(KT, 128, S)
        m = {
            "xt": xt,
            "wq": Wq[:, cs:cs + CW].astype(NP_BF16).reshape(KT, 128, CW),
            "wk": Wk[:, cs:cs + CW].astype(NP_BF16).reshape(KT, 128, CW),
            "wv": Wv[:, cs:cs + CW].astype(NP_BF16).reshape(KT, 128, CW),
            "wo": Wo[cs:cs + CW, :].astype(NP_BF16).reshape(2, 128, D),
            "bias": np.stack([bq[cs:cs + CW].reshape(2, 128),
                              bk[cs:cs + CW].reshape(2, 128)]).astype(np.float32),
        }
        in_maps.append(m)
    return in_maps


_NC_CACHE = {}


def _ensure_ntff_hook():
    """Register the axon NTFF profile hook if the image's antenv lacks it."""
    try:
        from antenv.axon_hooks import get_axon_ntff_profile_hook  # noqa: F401
        return  # already available
    except ImportError:
        pass
    try:
        import types
        import antenv
        from trn_agent_boot.trn_boot import _ntff_profile_via_ctypes
        hook = _ntff_profile_via_ctypes("/opt/axon/libaxon_pjrt.so")
        mod = types.ModuleType("antenv.axon_hooks")
        mod._hook = hook
        mod.get_axon_ntff_profile_hook = lambda: mod._hook
        mod.set_axon_ntff_profile_hook = lambda h: setattr(mod, "_hook", h)
        sys.modules["antenv.axon_hooks"] = mod
        antenv.axon_hooks = mod
    except Exception as e:  # profiling degrades, run still works
        print(f"ntff hook setup failed: {e}", file=sys.stderr)


def run_cores(in_maps, S=S_FULL, trace=False, trace_cores=None):
    if trace:
        _ensure_ntff_hook()
    if S not in _NC_CACHE:
        _NC_CACHE[S] = _build_full(S)
    nc = _NC_CACHE[S]
    return bass_utils.run_bass_kernel_spmd(
        nc, in_maps, core_ids=list(range(N_CORES)),
        trace=trace, trace_cores=trace_cores)


def kernel(X, Wq, bq, Wk, bk, Wv, bv, Wo, bo):
    X = np.asarray(X, dtype=np.float32)
    Wq, Wk, Wv, Wo = (np.asarray(w, dtype=np.float32) for w in (Wq, Wk, Wv, Wo))
    bq, bk, bv, bo = (np.asarray(v, dtype=np.float32) for v in (bq, bk, bv, bo))
    S = X.shape[1]
    in_maps = make_core_inputs(X, Wq, bq, Wk, bk, Wv, bv, Wo, bo, S=S)
    res = run_cores(in_maps, S=S)
    out = np.zeros((B, S, D), dtype=np.float32)
    for c in range(N_CORES):
        out[c // (N_CORES // B)] += np.asarray(res.results[c]["out"],
                                               dtype=np.float32)
    out += bo + bv @ Wo   # V bias commutes through softmax
    return out


# revision 14
# speedup vs baseline: 1.2399x; 1.0198x over previous
"""Multi-head self-attention on 8 TRN2 NeuronCores.

Full inputs in, full output out. Sharding: tensor-parallel over heads
(4 heads / core) x data-parallel over batch (cores 0-3 -> batch 0,
cores 4-7 -> batch 1). Each core computes a partial [S, D] output
through its 256-row slice of Wo; the host sums the 4 partials per batch
(row-parallel reduce) and adds bo + bv @ Wo (the V bias commutes
through softmax, so it's folded into the host-side bias).

Per-core dataflow (all matmuls bf16 with fp32 PSUM accumulation):
  - scores computed transposed (K @ Q^T) per HEAD PAIR: the two heads of
    a pair live in SBUF partitions 0-63 / 64-127, so their K=64 matmuls
    run CONCURRENTLY on disjoint PE row-tiles (64x128 tiling mode) into
    the two halves of one [128, 1024] PSUM tile.
  - one exp activation per step covers both heads ([128,1024], PSUM src,
    scale=1/sqrt(hd) and a constant -4 bias fused; no max-subtraction)
  - V stored with a fused ones-column per head ([V_h | 1] -> M=65 AV
    matmuls compute attn^T and the softmax denominator in one chain)
  - attention runs as a flat 128-step pipeline (8 blocks = 2 pairs x 4
    query windows of 512, x 16 key chunks); AV trails scores by 2 steps
    so the Scalar engine (the ~136us floor) never bubbles; projection
    chains (QKV, V, output) are slotted into PE slack with explicit
    deadlines.
  - output projection: attn_concat^T stationary, Wo moving, K=256;
    each window staged in SBUF bf16 and shipped with one DMA.
"""

import os
import sys
from contextlib import ExitStack

import numpy as np
import ml_dtypes

sys.path.insert(0, "/opt/trn_rl_repo")

import concourse.bass as bass
import concourse.tile as tile
from concourse import bacc, mybir
from concourse import bass_utils

BF16 = mybir.dt.bfloat16
FP32 = mybir.dt.float32
NP_BF16 = ml_dtypes.bfloat16

D = 1024          # d_model
H = 16            # total heads
HD = 64           # head dim
B = 2             # batch
S_FULL = 2048     # sequence length
N_CORES = 8
HPC = 4           # heads per core
CW = HPC * HD     # 256 per-core qkv columns
KT = D // 128     # 8 contraction tiles
W = 512           # query window per block
NW = S_FULL // W  # 4 windows
ST = S_FULL // 128  # 16 key chunks


def build_mha_kernel(ctx: ExitStack, tc: "tile.TileContext", out_ap: bass.AP,
                     ins: dict, S: int = S_FULL):
    """Emit the per-core MHA kernel body.

    ins: dict of APs: xt [KT,128,S] bf16, wq/wk/wv [KT,128,CW] bf16,
         wo [2,128,D] bf16, bias [2,2,128] f32.
    out_ap: [S, D] bf16 partial output (no biases).
    """
    assert S == S_FULL
    nc = tc.nc
    xt_d, wq_d, wk_d, wv_d = ins["xt"], ins["wq"], ins["wk"], ins["wv"]
    wo_d, bias_d = ins["wo"], ins["bias"]

    SCALE = 1.0 / np.sqrt(HD)
    EXP_BIAS = -4.0               # constant shift; cancels in softmax

    const = ctx.enter_context(tc.tile_pool(name="const", bufs=1))

    # ---- persistent SBUF tensors ----
    xt_sb = const.tile([128, KT * S], BF16, tag="xt", name="xt_sb")
    wq_sb = const.tile([128, KT * CW], BF16, tag="wq", name="wq_sb")
    wk_sb = const.tile([128, KT * CW], BF16, tag="wk", name="wk_sb")
    wv_sb = const.tile([128, KT * CW], BF16, tag="wv", name="wv_sb")
    wo_sb = const.tile([128, 2 * D], BF16, tag="wo", name="wo_sb")
    bias_sb = const.tile([128, 4], FP32, tag="bias", name="bias_sb")
    qt_sb = [const.tile([128, S], BF16, tag=f"qt{i}", name=f"qt_sb{i}") for i in range(2)]
    kt_sb = [const.tile([128, S], BF16, tag=f"kt{i}", name=f"kt_sb{i}") for i in range(2)]
    # V with a ones column per head: [V_h0 |1| V_h1 |1| V_h2 |1| V_h3 |1]
    vaug_sb = [const.tile([128, HPC * 65], BF16, tag=f"v{t}", name=f"vaug_sb{t}") for t in range(ST)]
    atT_sb = [const.tile([128, S], BF16, tag=f"at{i}", name=f"atT_sb{i}") for i in range(2)]

    # ---- input DMAs: few coarse kicks, priority-ordered so the first
    # chains (wk, wq, xt cols 0-511) are satisfied earliest ----
    def load_w(sb, dram):
        nc.sync.dma_start(out=sb[:].rearrange("p (k c) -> p k c", k=KT),
                          in_=dram.rearrange("k p c -> p k c"))

    def load_xt_quarter(q, k0, k1):
        qs, qe = q * 512, (q + 1) * 512
        nc.sync.dma_start(
            out=xt_sb[:].rearrange("p (k c) -> p k c", k=KT)[:, k0:k1, qs:qe],
            in_=xt_d[k0:k1, :, qs:qe].rearrange("k p c -> p k c"))

    def load_w_pair(sb, dram, ht):
        nc.sync.dma_start(
            out=sb[:].rearrange("p (k c) -> p k c", k=KT)[:, :, ht * 128:(ht + 1) * 128],
            in_=dram[:, :, ht * 128:(ht + 1) * 128].rearrange("k p c -> p k c"))

    nc.sync.dma_start(out=bias_sb[:].rearrange("p (m t) -> p m t", m=2),
                      in_=bias_d.rearrange("m t p -> p m t"))
    load_w_pair(wk_sb, wk_d, 0)
    load_w_pair(wq_sb, wq_d, 0)
    load_xt_quarter(0, 0, 4)
    load_xt_quarter(0, 4, 8)
    load_w(wv_sb, wv_d)
    load_xt_quarter(1, 0, 8)
    load_w_pair(wk_sb, wk_d, 1)
    load_w_pair(wq_sb, wq_d, 1)
    load_xt_quarter(2, 0, 8)
    load_xt_quarter(3, 0, 8)
    nc.sync.dma_start(out=wo_sb[:].rearrange("p (k c) -> p k c", k=2),
                      in_=wo_d.rearrange("k p c -> p k c"))

    # ones columns of vaug
    for t in range(ST):
        nc.vector.memset(
            vaug_sb[t][:].rearrange("p (h c) -> p h c", c=65)[:, :, 64:65], 1.0)

    # per-partition constant bias for the exp activation
    ebias_sb = const.tile([128, 1], FP32, tag="ebias", name="ebias_sb")
    nc.vector.memset(ebias_sb[:], EXP_BIAS)

    # ---- PSUM pools: 4 + 2 + 1 + 1 = 8 banks ----
    # qk chains hold their psum tile across two emission steps, so they get
    # a dedicated single-buffer pool; V/outproj/warmup tiles are transient
    # (outproj ping-pongs between the two single-buffer pools).
    sc_psum = ctx.enter_context(tc.tile_pool(name="sc", bufs=2, space="PSUM"))
    av_psum = ctx.enter_context(tc.tile_pool(name="av", bufs=2, space="PSUM"))
    pjq_psum = ctx.enter_context(tc.tile_pool(name="pjq", bufs=1, space="PSUM"))
    pjv_psum = ctx.enter_context(tc.tile_pool(name="pjv", bufs=1, space="PSUM"))

    exp_pool = ctx.enter_context(tc.tile_pool(name="expp", bufs=6))
    fin_pool = ctx.enter_context(tc.tile_pool(name="fin", bufs=2))
    ostw_pool = ctx.enter_context(tc.tile_pool(name="ostw", bufs=2))

    # ---- projection emitters ----
    qk_pending = {}   # (mi, ht, c) -> psum tile across the two halves

    def emit_qk_half(mi, ht, c, half):
        """Half (4 k-tiles) of a Q^T (mi=0) / K^T (mi=1) chain for pair ht,
        columns [c*512, (c+1)*512)."""
        w_sb, dst = ((wq_sb, qt_sb), (wk_sb, kt_sb))[mi]
        key = (mi, ht, c)
        if half == 0:
            qk_pending[key] = pjq_psum.tile([128, 512], FP32, tag="pjq", name="pjqt")
        pt = qk_pending[key]
        for k in range(4 * half, 4 * half + 4):
            nc.tensor.matmul(
                pt[:],
                lhsT=w_sb[:, k * CW + ht * 128: k * CW + ht * 128 + 128],
                rhs=xt_sb[:, k * S + c * 512: k * S + c * 512 + 512],
                start=(k == 0), stop=(k == KT - 1))
        if half == 1:
            del qk_pending[key]
            nc.vector.tensor_scalar_add(
                out=dst[ht][:, c * 512: c * 512 + 512],
                in0=pt[:],
                scalar1=bias_sb[:, mi * 2 + ht: mi * 2 + ht + 1])

    def emit_v_pair(pr, t, pool=None, tag="pj"):
        """V natural rows t*128..+128 for head pair pr (2 heads)."""
        pt = (pool or pjv_psum).tile([128, 512], FP32, tag=tag, name="pjt")
        for k in range(KT):
            nc.tensor.matmul(
                pt[:, 0:128],
                lhsT=xt_sb[:, k * S + t * 128: k * S + t * 128 + 128],
                rhs=wv_sb[:, k * CW + pr * 128: k * CW + pr * 128 + 128],
                start=(k == 0), stop=(k == KT - 1))
        for hh in range(2):
            h = pr * 2 + hh
            nc.vector.tensor_copy(
                out=vaug_sb[t][:, h * 65: h * 65 + 64],
                in_=pt[:, hh * 64: hh * 64 + 64])

    ow_state = {}   # window -> staging tile

    def emit_outproj_unit(wdx, u):
        """One outproj unit: rows (4*wdx + u//2)*128..+128, cols (u%2)*512."""
        st, nch = wdx * 4 + u // 2, u % 2
        pool = pjv_psum if u % 2 == 0 else pjq_psum
        pt = pool.tile([128, 512], FP32, tag="pj" if u % 2 == 0 else "pjq",
                       name="pjt")
        for k2 in range(2):
            nc.tensor.matmul(
                pt[:],
                lhsT=atT_sb[k2][:, st * 128: st * 128 + 128],
                rhs=wo_sb[:, k2 * D + nch * 512: k2 * D + nch * 512 + 512],
                start=(k2 == 0), stop=(k2 == 1))
        if wdx not in ow_state:
            ow_state[wdx] = ostw_pool.tile([128, 4 * D], BF16, tag="ostw",
                                           name="ostw")
        stg = ow_state[wdx]
        nc.vector.tensor_copy(
            out=stg[:, (st % 4) * D + nch * 512: (st % 4) * D + nch * 512 + 512],
            in_=pt[:])
        if u == 3 or u == 7:   # half-window complete -> DMA
            h0 = 0 if u == 3 else 2
            if u == 7:
                del ow_state[wdx]
            nc.sync.dma_start(
                out=out_ap.rearrange("(st p) d -> p st d", p=128)[
                    :, 4 * wdx + h0: 4 * wdx + h0 + 2, :],
                in_=stg[:, h0 * D: (h0 + 2) * D].rearrange("p (st d) -> p st d", st=2))

    # ---- PE warmup: junk matmuls bridging engine start -> first DMA pieces ----
    wsrc = const.tile([128, 512], BF16, tag="wsrc", name="wsrc")
    nc.vector.memset(wsrc[:], 0.25)
    wup = pjv_psum.tile([128, 512], FP32, tag="pj", name="wup")
    for i in range(20):
        nc.tensor.matmul(wup[:], lhsT=wsrc[:, 0:128], rhs=wsrc[:],
                         start=(i == 0), stop=(i == 19))

    # ---- lead-in chains: exactly what step 0 needs ----
    emit_qk_half(1, 0, 0, 0)
    emit_qk_half(1, 0, 0, 1)   # K^T pair0 cols 0-511 (key chunks 0-3)
    emit_qk_half(0, 0, 0, 0)
    emit_qk_half(0, 0, 0, 1)   # Q^T pair0 cols 0-511 (window 0)
    emit_v_pair(0, 0, pool=pjv_psum)
    emit_v_pair(0, 1, pool=pjq_psum, tag="pjq")

    # ---- per-step work schedule (chains + outproj slotted into PE slack) ----
    work = {s: [] for s in range(130)}

    def addw(s, fn, *a):
        work[s].append((fn, a))

    # block 0 (pair0, window 0): V pair0 JIT + remaining K chains + Q(0,1)
    for s in range(14):
        addw(s, emit_v_pair, 0, s + 2)
    for ci, (s0, s1) in enumerate(((0, 2), (4, 6), (8, 10))):
        addw(s0, emit_qk_half, 1, 0, ci + 1, 0)
        addw(s1, emit_qk_half, 1, 0, ci + 1, 1)
    addw(12, emit_qk_half, 0, 0, 1, 0)
    addw(14, emit_qk_half, 0, 0, 1, 1)
    # blocks 1-2: remaining pair-0 Q chains + pair-1 K chains
    addw(16, emit_qk_half, 0, 0, 2, 0)
    addw(18, emit_qk_half, 0, 0, 2, 1)
    addw(20, emit_qk_half, 1, 1, 0, 0)
    addw(22, emit_qk_half, 1, 1, 0, 1)
    addw(24, emit_qk_half, 1, 1, 1, 0)
    addw(26, emit_qk_half, 1, 1, 1, 1)
    addw(32, emit_qk_half, 0, 0, 3, 0)
    addw(34, emit_qk_half, 0, 0, 3, 1)
    addw(36, emit_qk_half, 1, 1, 2, 0)
    addw(38, emit_qk_half, 1, 1, 2, 1)
    addw(40, emit_qk_half, 1, 1, 3, 0)
    addw(42, emit_qk_half, 1, 1, 3, 1)
    # blocks 3-4 (light): pair-1 V chains on odd steps + pair-1 Q chains
    addw(48, emit_qk_half, 0, 1, 0, 0)
    addw(50, emit_qk_half, 0, 1, 0, 1)
    for i in range(8):
        addw(49 + 2 * i, emit_v_pair, 1, i)
    addw(72, emit_qk_half, 0, 1, 1, 0)
    addw(74, emit_qk_half, 0, 1, 1, 1)
    for i in range(8):
        addw(65 + 2 * i, emit_v_pair, 1, 8 + i)
    # blocks 5-7: remaining pair-1 Q chains + outproj of completed windows.
    # window w complete after block 4+w; its finalize lands at step 81+16w.
    addw(80, emit_qk_half, 0, 1, 2, 0)
    addw(82, emit_qk_half, 0, 1, 2, 1)
    addw(96, emit_qk_half, 0, 1, 3, 0)
    addw(98, emit_qk_half, 0, 1, 3, 1)
    for wdx, base in ((0, 84), (1, 100), (2, 116)):
        for u in range(8):
            addw(base + u, emit_outproj_unit, wdx, u)

    def finalize_pair(ht, jw, avp, last=False):
        """Normalize both heads of pair ht for query window jw.

        attn^T_unnorm -> atT (bf16), then scaled by the broadcast
        reciprocal of the denominator row. No bias (folded into the
        host-side reduce as bv @ Wo). The mid-run variant copies the
        raw attn out first so the AV psum frees early; the `last`
        variant shortens the critical path by multiplying straight
        from PSUM (nothing trails it, so the late psum release is ok).
        """
        dsts, dens, recs = [], [], []
        for hh in range(2):
            hr = hh * 64
            dst = atT_sb[ht][hr: hr + 64, jw * W: (jw + 1) * W]
            den = fin_pool.tile([1, W], FP32, tag=f"den{hh}", name="den")
            nc.vector.tensor_copy(den[:], avp[hh][64:65, :])
            if not last:
                nc.vector.tensor_copy(dst, avp[hh][0:64, :])
            dsts.append(dst)
            dens.append(den)
        for hh in range(2):
            rec = fin_pool.tile([1, W], FP32, tag=f"rec{hh}", name="rec")
            nc.vector.reciprocal_approx_fast(out=rec[:], in_=dens[hh][:])
            recs.append(rec)
        bcs = []
        for hh in range(2):
            bc = fin_pool.tile([128, W], FP32, tag=f"bc{hh}", name="bc")
            nc.gpsimd.partition_broadcast(bc[:], recs[hh][:])
            bcs.append(bc)
        for hh in range(2):
            hr = hh * 64
            if last:
                nc.vector.tensor_mul(dsts[hh], avp[hh][0:64, :],
                                     bcs[hh][hr: hr + 64, :])
            else:
                nc.vector.tensor_mul(dsts[hh], dsts[hh], bcs[hh][hr: hr + 64, :])

    # ---- attention: flat 128-step pipeline, AV trailing scores by 2 ----
    pending = {}    # step -> ex tile
    block_av = {}   # block -> (av_h0, av_h1)
    for step in range(130):
        if step < 128:
            b, t = step // 16, step % 16
            ht, jw = b // 4, b % 4
            if t == 0:
                block_av[b] = tuple(
                    av_psum.tile([65, W], FP32, tag="av", name="avt")
                    for _ in range(2))
            sc = sc_psum.tile([128, 2 * W], FP32, tag="sc", name="sct")
            for hh in range(2):
                hr = hh * 64
                nc.tensor.matmul(
                    sc[:, hh * W: (hh + 1) * W],
                    lhsT=kt_sb[ht][hr: hr + 64, t * 128: t * 128 + 128],
                    rhs=qt_sb[ht][hr: hr + 64, jw * W: (jw + 1) * W],
                    start=True, stop=True)
            ex = exp_pool.tile([128, 2 * W], BF16, tag="exp", name="ex")
            nc.scalar.activation(ex[:], sc[:],
                                 mybir.ActivationFunctionType.Exp,
                                 bias=ebias_sb[:], scale=SCALE)
            pending[step] = ex
        for fn, a in work[step]:
            fn(*a)
        ps = step - 2
        if ps >= 0:
            ex = pending.pop(ps)
            b2, t2 = ps // 16, ps % 16
            ht2, jw2 = b2 // 4, b2 % 4
            for hh in range(2):
                h = ht2 * 2 + hh
                nc.tensor.matmul(
                    block_av[b2][hh][:],
                    lhsT=vaug_sb[t2][:, h * 65: h * 65 + 65],
                    rhs=ex[:, hh * W: (hh + 1) * W],
                    start=(t2 == 0), stop=(t2 == ST - 1))
            if t2 == ST - 1:
                finalize_pair(ht2, jw2, block_av.pop(b2), last=(b2 == 7))

    # ---- tail: junk matmuls keep HAM warm while the last finalize's
    # DVE/GpSimd chain runs, then the last window's output projection ----
    wup2 = pjv_psum.tile([128, 512], FP32, tag="pj", name="wup2")
    for i in range(12):
        nc.tensor.matmul(wup2[:], lhsT=wsrc[:, 0:128], rhs=wsrc[:],
                         start=(i == 0), stop=(i == 11))
    for u in range(8):
        emit_outproj_unit(3, u)


def _build_full(S=S_FULL):
    nc = bacc.Bacc("TRN2", target_bir_lowering=False, debug=False,
                   num_devices=N_CORES)
    xt = nc.dram_tensor("xt", [KT, 128, S], BF16, kind="ExternalInput")
    wq = nc.dram_tensor("wq", [KT, 128, CW], BF16, kind="ExternalInput")
    wk = nc.dram_tensor("wk", [KT, 128, CW], BF16, kind="ExternalInput")
    wv = nc.dram_tensor("wv", [KT, 128, CW], BF16, kind="ExternalInput")
    wo = nc.dram_tensor("wo", [2, 128, D], BF16, kind="ExternalInput")
    bias = nc.dram_tensor("bias", [2, 2, 128], FP32, kind="ExternalInput")
    out = nc.dram_tensor("out", [S, D], BF16, kind="ExternalOutput")
    ins = {k: t.ap() for k, t in
           dict(xt=xt, wq=wq, wk=wk, wv=wv, wo=wo, bias=bias).items()}
    with tile.TileContext(nc) as tc:
        with ExitStack() as ctx:
            build_mha_kernel(ctx, tc, out.ap(), ins, S=S)
    nc.compile()
    return nc


def make_core_inputs(X, Wq, bq, Wk, bk, Wv, bv, Wo, bo, S=S_FULL):
    """Build the 8 per-core input maps (host-side shard + transpose + cast)."""
    in_maps = []
    for c in range(N_CORES):
        b = c // (N_CORES // B)
        cs = (c % (N_CORES // B)) * CW
        xt = np.ascontiguousarray(X[b].T).astype(NP_BF16).reshape(KT, 128, S)
        m = {
            "xt": xt,
            "wq": Wq[:, cs:cs + CW].astype(NP_BF16).reshape(KT, 128, CW),
            "wk": Wk[:, cs:cs + CW].astype(NP_BF16).reshape(KT, 128, CW),
            "wv": Wv[:, cs:cs + CW].astype(NP_BF16).reshape(KT, 128, CW),
            "wo": Wo[cs:cs + CW, :].astype(NP_BF16).reshape(2, 128, D),
            "bias": np.stack([bq[cs:cs + CW].reshape(2, 128),
                              bk[cs:cs + CW].reshape(2, 128)]).astype(np.float32),
        }
        in_maps.append(m)
    return in_maps


_NC_CACHE = {}


def _ensure_ntff_hook():
    """Register the axon NTFF profile hook if the image's antenv lacks it."""
    try:
        from antenv.axon_hooks import get_axon_ntff_profile_hook  # noqa: F401
        return  # already available
    except ImportError:
        pass
    try:
        import types
        import antenv
        from trn_agent_boot.trn_boot import _ntff_profile_via_ctypes
        hook = _ntff_profile_via_ctypes("/opt/axon/libaxon_pjrt.so")
        mod = types.ModuleType("antenv.axon_hooks")
        mod._hook = hook
        mod.get_axon_ntff_profile_hook = lambda: mod._hook
        mod.set_axon_ntff_profile_hook = lambda h: setattr(mod, "_hook", h)
        sys.modules["antenv.axon_hooks"] = mod
        antenv.axon_hooks = mod
    except Exception as e:  # profiling degrades, run still works
        print(f"ntff hook setup failed: {e}", file=sys.stderr)


def run_cores(in_maps, S=S_FULL, trace=False, trace_cores=None):
    if trace:
        _ensure_ntff_hook()
    if S not in _NC_CACHE:
        _NC_CACHE[S] = _build_full(S)
    nc = _NC_CACHE[S]
    return bass_utils.run_bass_kernel_spmd(
        nc, in_maps, core_ids=list(range(N_CORES)),
        trace=trace, trace_cores=trace_cores)


def kernel(X, Wq, bq, Wk, bk, Wv, bv, Wo, bo):
    X = np.asarray(X, dtype=np.float32)
    Wq, Wk, Wv, Wo = (np.asarray(w, dtype=np.float32) for w in (Wq, Wk, Wv, Wo))
    bq, bk, bv, bo = (np.asarray(v, dtype=np.float32) for v in (bq, bk, bv, bo))
    S = X.shape[1]
    in_maps = make_core_inputs(X, Wq, bq, Wk, bk, Wv, bv, Wo, bo, S=S)
    res = run_cores(in_maps, S=S)
    out = np.zeros((B, S, D), dtype=np.float32)
    for c in range(N_CORES):
        out[c // (N_CORES // B)] += np.asarray(res.results[c]["out"],
                                               dtype=np.float32)
    out += bo + bv @ Wo   # V bias commutes through softmax
    return out


# revision 23
# speedup vs baseline: 1.2401x; 1.0002x over previous
"""Multi-head self-attention on 8 TRN2 NeuronCores.

Full inputs in, full output out. Sharding: tensor-parallel over heads
(4 heads / core) x data-parallel over batch (cores 0-3 -> batch 0,
cores 4-7 -> batch 1). Each core computes a partial [S, D] output
through its 256-row slice of Wo; the host sums the 4 partials per batch
(row-parallel reduce) and adds bo + bv @ Wo (the V bias commutes
through softmax, so it's folded into the host-side bias).

Per-core dataflow (all matmuls bf16 with fp32 PSUM accumulation):
  - scores computed transposed (K @ Q^T) per HEAD PAIR: the two heads of
    a pair live in SBUF partitions 0-63 / 64-127, so their K=64 matmuls
    run CONCURRENTLY on disjoint PE row-tiles (64x128 tiling mode) into
    the two halves of one [128, 1024] PSUM tile.
  - one exp activation per step covers both heads ([128,1024], PSUM src,
    scale=1/sqrt(hd) and a constant -4 bias fused; no max-subtraction)
  - V stored with a fused ones-column per head ([V_h | 1] -> M=65 AV
    matmuls compute attn^T and the softmax denominator in one chain)
  - attention runs as a flat 128-step pipeline (8 blocks = 2 pairs x 4
    query windows of 512, x 16 key chunks); AV trails scores by 2 steps
    so the Scalar engine (the ~136us floor) never bubbles; projection
    chains (QKV, V, output) are slotted into PE slack with explicit
    deadlines.
  - output projection: attn_concat^T stationary, Wo moving, K=256;
    each window staged in SBUF bf16 and shipped with one DMA.
"""

import os
import sys
from contextlib import ExitStack

import numpy as np
import ml_dtypes

sys.path.insert(0, "/opt/trn_rl_repo")

import concourse.bass as bass
import concourse.tile as tile
from concourse import bacc, mybir
from concourse import bass_utils

BF16 = mybir.dt.bfloat16
FP32 = mybir.dt.float32
NP_BF16 = ml_dtypes.bfloat16

D = 1024          # d_model
H = 16            # total heads
HD = 64           # head dim
B = 2             # batch
S_FULL = 2048     # sequence length
N_CORES = 8
HPC = 4           # heads per core
CW = HPC * HD     # 256 per-core qkv columns
KT = D // 128     # 8 contraction tiles
W = 512           # query window per block
NW = S_FULL // W  # 4 windows
ST = S_FULL // 128  # 16 key chunks


def build_mha_kernel(ctx: ExitStack, tc: "tile.TileContext", out_ap: bass.AP,
                     ins: dict, S: int = S_FULL):
    """Emit the per-core MHA kernel body.

    ins: dict of APs: xt [KT,128,S] bf16, wq/wk/wv [KT,128,CW] bf16,
         wo [2,128,D] bf16, bias [2,2,128] f32.
    out_ap: [S, D] bf16 partial output (no biases).
    """
    assert S == S_FULL
    nc = tc.nc
    xt_d, wq_d, wk_d, wv_d = ins["xt"], ins["wq"], ins["wk"], ins["wv"]
    wo_d, bias_d = ins["wo"], ins["bias"]

    SCALE = 1.0 / np.sqrt(HD)
    EXP_BIAS = -4.0               # constant shift; cancels in softmax

    const = ctx.enter_context(tc.tile_pool(name="const", bufs=1))

    # ---- persistent SBUF tensors ----
    xt_sb = const.tile([128, KT * S], BF16, tag="xt", name="xt_sb")
    wq_sb = const.tile([128, KT * CW], BF16, tag="wq", name="wq_sb")
    wk_sb = const.tile([128, KT * CW], BF16, tag="wk", name="wk_sb")
    wv_sb = const.tile([128, KT * CW], BF16, tag="wv", name="wv_sb")
    wo_sb = const.tile([128, 2 * D], BF16, tag="wo", name="wo_sb")
    bias_sb = const.tile([128, 4], FP32, tag="bias", name="bias_sb")
    qt_sb = [const.tile([128, S], BF16, tag=f"qt{i}", name=f"qt_sb{i}") for i in range(2)]
    kt_sb = [const.tile([128, S], BF16, tag=f"kt{i}", name=f"kt_sb{i}") for i in range(2)]
    # V with a ones column per head: [V_h0 |1| V_h1 |1| V_h2 |1| V_h3 |1]
    vaug_sb = [const.tile([128, HPC * 65], BF16, tag=f"v{t}", name=f"vaug_sb{t}") for t in range(ST)]
    atT_sb = [const.tile([128, S], BF16, tag=f"at{i}", name=f"atT_sb{i}") for i in range(2)]

    # ---- input DMAs: host pre-packs every tensor into its SBUF layout, so
    # each load is a contiguous 2D copy (4-8KB lines, near-peak DMA bw).
    # Priority order: the first chains need wk, xt quarter 0, wq.
    # xt is quarter-major in DRAM: [q][p][k][c], SBUF col = q*4096+k*512+c.
    def xcol(k, s):
        return (s // 512) * (KT * 512) + k * 512 + (s % 512)

    nc.sync.dma_start(out=bias_sb[:].rearrange("p (m t) -> p m t", m=2),
                      in_=bias_d.rearrange("m t p -> p m t"))
    nc.sync.dma_start(out=wk_sb[:], in_=wk_d)
    nc.sync.dma_start(
        out=xt_sb[:, 0:KT * 512],
        in_=xt_d[0:1].rearrange("q p k c -> p (q k c)"))
    nc.sync.dma_start(out=wq_sb[:], in_=wq_d)
    nc.sync.dma_start(
        out=xt_sb[:, KT * 512: 2 * KT * 512],
        in_=xt_d[1:2].rearrange("q p k c -> p (q k c)"))
    nc.sync.dma_start(out=wv_sb[:], in_=wv_d)
    for q in (2, 3):
        nc.sync.dma_start(
            out=xt_sb[:, q * KT * 512:(q + 1) * KT * 512],
            in_=xt_d[q:q + 1].rearrange("q p k c -> p (q k c)"))
    nc.sync.dma_start(out=wo_sb[:], in_=wo_d)

    # ones columns of vaug
    for t in range(ST):
        nc.vector.memset(
            vaug_sb[t][:].rearrange("p (h c) -> p h c", c=65)[:, :, 64:65], 1.0)

    # per-partition constant bias for the exp activation
    ebias_sb = const.tile([128, 1], FP32, tag="ebias", name="ebias_sb")
    nc.vector.memset(ebias_sb[:], EXP_BIAS)

    # ---- PSUM pools: 4 + 2 + 1 + 1 = 8 banks ----
    # qk chains hold their psum tile across two emission steps, so they get
    # a dedicated single-buffer pool; V/outproj/warmup tiles are transient
    # (outproj ping-pongs between the two single-buffer pools).
    sc_psum = ctx.enter_context(tc.tile_pool(name="sc", bufs=2, space="PSUM"))
    av_psum = ctx.enter_context(tc.tile_pool(name="av", bufs=2, space="PSUM"))
    pjq_psum = ctx.enter_context(tc.tile_pool(name="pjq", bufs=1, space="PSUM"))
    pjv_psum = ctx.enter_context(tc.tile_pool(name="pjv", bufs=1, space="PSUM"))

    exp_pool = ctx.enter_context(tc.tile_pool(name="expp", bufs=6))
    fin_pool = ctx.enter_context(tc.tile_pool(name="fin", bufs=2))
    ostw_pool = ctx.enter_context(tc.tile_pool(name="ostw", bufs=2))

    # ---- projection emitters ----
    qk_pending = {}   # (mi, ht, c) -> psum tile across the two halves

    def emit_qk_half(mi, ht, c, half):
        """Half (4 k-tiles) of a Q^T (mi=0) / K^T (mi=1) chain for pair ht,
        columns [c*512, (c+1)*512)."""
        w_sb, dst = ((wq_sb, qt_sb), (wk_sb, kt_sb))[mi]
        key = (mi, ht, c)
        if half == 0:
            qk_pending[key] = pjq_psum.tile([128, 512], FP32, tag="pjq", name="pjqt")
        pt = qk_pending[key]
        for k in range(4 * half, 4 * half + 4):
            nc.tensor.matmul(
                pt[:],
                lhsT=w_sb[:, k * CW + ht * 128: k * CW + ht * 128 + 128],
                rhs=xt_sb[:, xcol(k, c * 512): xcol(k, c * 512) + 512],
                start=(k == 0), stop=(k == KT - 1))
        if half == 1:
            del qk_pending[key]
            nc.vector.tensor_scalar_add(
                out=dst[ht][:, c * 512: c * 512 + 512],
                in0=pt[:],
                scalar1=bias_sb[:, mi * 2 + ht: mi * 2 + ht + 1])

    def emit_v_pair(pr, t, pool=None, tag="pj"):
        """V natural rows t*128..+128 for head pair pr (2 heads)."""
        pt = (pool or pjv_psum).tile([128, 512], FP32, tag=tag, name="pjt")
        for k in range(KT):
            nc.tensor.matmul(
                pt[:, 0:128],
                lhsT=xt_sb[:, xcol(k, t * 128): xcol(k, t * 128) + 128],
                rhs=wv_sb[:, k * CW + pr * 128: k * CW + pr * 128 + 128],
                start=(k == 0), stop=(k == KT - 1))
        for hh in range(2):
            h = pr * 2 + hh
            nc.vector.tensor_copy(
                out=vaug_sb[t][:, h * 65: h * 65 + 64],
                in_=pt[:, hh * 64: hh * 64 + 64])

    ow_state = {}   # window -> staging tile

    def emit_outproj_unit(wdx, u, use_act=False):
        """One outproj unit: rows (4*wdx + u//2)*128..+128, cols (u%2)*512."""
        st, nch = wdx * 4 + u // 2, u % 2
        pool = pjv_psum if u % 2 == 0 else pjq_psum
        pt = pool.tile([128, 512], FP32, tag="pj" if u % 2 == 0 else "pjq",
                       name="pjt")
        for k2 in range(2):
            nc.tensor.matmul(
                pt[:],
                lhsT=atT_sb[k2][:, st * 128: st * 128 + 128],
                rhs=wo_sb[:, k2 * D + nch * 512: k2 * D + nch * 512 + 512],
                start=(k2 == 0), stop=(k2 == 1))
        if wdx not in ow_state:
            ow_state[wdx] = ostw_pool.tile([128, 4 * D], BF16, tag="ostw",
                                           name="ostw")
        stg = ow_state[wdx]
        dst = stg[:, (st % 4) * D + nch * 512: (st % 4) * D + nch * 512 + 512]
        if use_act:
            nc.scalar.copy(out=dst, in_=pt[:])
        else:
            nc.vector.tensor_copy(out=dst, in_=pt[:])
        if u == 3 or u == 7:   # half-window complete -> DMA
            h0 = 0 if u == 3 else 2
            if u == 7:
                del ow_state[wdx]
            nc.sync.dma_start(
                out=out_ap.rearrange("(st p) d -> p st d", p=128)[
                    :, 4 * wdx + h0: 4 * wdx + h0 + 2, :],
                in_=stg[:, h0 * D: (h0 + 2) * D].rearrange("p (st d) -> p st d", st=2))

    # ---- PE warmup: junk matmuls bridging engine start -> first DMA pieces ----
    wsrc = const.tile([128, 512], BF16, tag="wsrc", name="wsrc")
    nc.vector.memset(wsrc[:], 0.25)
    wup = pjv_psum.tile([128, 512], FP32, tag="pj", name="wup")
    for i in range(20):
        nc.tensor.matmul(wup[:], lhsT=wsrc[:, 0:128], rhs=wsrc[:],
                         start=(i == 0), stop=(i == 19))

    # ---- lead-in chains: exactly what step 0 needs ----
    emit_qk_half(1, 0, 0, 0)
    emit_qk_half(1, 0, 0, 1)   # K^T pair0 cols 0-511 (key chunks 0-3)
    emit_qk_half(0, 0, 0, 0)
    emit_qk_half(0, 0, 0, 1)   # Q^T pair0 cols 0-511 (window 0)

    # ---- per-step work schedule (chains + outproj slotted into PE slack) ----
    work = {s: [] for s in range(130)}

    def addw(s, fn, *a):
        work[s].append((fn, a))

    # block 0 (pair0, window 0): V pair0 JIT + remaining K chains + Q(0,1)
    for s in range(16):
        addw(s, emit_v_pair, 0, s)
    for ci, (s0, s1) in enumerate(((0, 2), (4, 6), (8, 10))):
        addw(s0, emit_qk_half, 1, 0, ci + 1, 0)
        addw(s1, emit_qk_half, 1, 0, ci + 1, 1)
    addw(12, emit_qk_half, 0, 0, 1, 0)
    addw(14, emit_qk_half, 0, 0, 1, 1)
    # blocks 1-2: remaining pair-0 Q chains + pair-1 K chains
    addw(16, emit_qk_half, 0, 0, 2, 0)
    addw(18, emit_qk_half, 0, 0, 2, 1)
    addw(20, emit_qk_half, 1, 1, 0, 0)
    addw(22, emit_qk_half, 1, 1, 0, 1)
    addw(24, emit_qk_half, 1, 1, 1, 0)
    addw(26, emit_qk_half, 1, 1, 1, 1)
    addw(32, emit_qk_half, 0, 0, 3, 0)
    addw(34, emit_qk_half, 0, 0, 3, 1)
    addw(36, emit_qk_half, 1, 1, 2, 0)
    addw(38, emit_qk_half, 1, 1, 2, 1)
    addw(40, emit_qk_half, 1, 1, 3, 0)
    addw(42, emit_qk_half, 1, 1, 3, 1)
    # blocks 3-4 (light): pair-1 V chains on odd steps + pair-1 Q chains
    addw(48, emit_qk_half, 0, 1, 0, 0)
    addw(50, emit_qk_half, 0, 1, 0, 1)
    for i in range(8):
        addw(49 + 2 * i, emit_v_pair, 1, i)
    addw(72, emit_qk_half, 0, 1, 1, 0)
    addw(74, emit_qk_half, 0, 1, 1, 1)
    for i in range(8):
        addw(65 + 2 * i, emit_v_pair, 1, 8 + i)
    # blocks 5-7: remaining pair-1 Q chains + outproj of completed windows.
    # window w complete after block 4+w; its finalize lands at step 81+16w.
    addw(80, emit_qk_half, 0, 1, 2, 0)
    addw(82, emit_qk_half, 0, 1, 2, 1)
    addw(96, emit_qk_half, 0, 1, 3, 0)
    addw(98, emit_qk_half, 0, 1, 3, 1)
    for wdx, base in ((0, 84), (1, 100), (2, 116)):
        for u in range(8):
            addw(base + u, emit_outproj_unit, wdx, u)

    def finalize_pair(ht, jw, avp, last=False):
        """Normalize both heads of pair ht for query window jw.

        attn^T_unnorm -> atT (bf16), then scaled by the broadcast
        reciprocal of the denominator row. No bias (folded into the
        host-side reduce as bv @ Wo). The mid-run variant copies the
        raw attn out first so the AV psum frees early; the `last`
        variant shortens the critical path by multiplying straight
        from PSUM (nothing trails it, so the late psum release is ok).
        """
        dsts, dens, recs = [], [], []
        for hh in range(2):
            hr = hh * 64
            dst = atT_sb[ht][hr: hr + 64, jw * W: (jw + 1) * W]
            den = fin_pool.tile([1, W], FP32, tag=f"den{hh}", name="den")
            nc.vector.tensor_copy(den[:], avp[hh][64:65, :])
            if not last:
                nc.vector.tensor_copy(dst, avp[hh][0:64, :])
            dsts.append(dst)
            dens.append(den)
        for hh in range(2):
            rec = fin_pool.tile([1, W], FP32, tag=f"rec{hh}", name="rec")
            nc.vector.reciprocal_approx_fast(out=rec[:], in_=dens[hh][:])
            recs.append(rec)
        bcs = []
        for hh in range(2):
            bc = fin_pool.tile([128, W], FP32, tag=f"bc{hh}", name="bc")
            nc.gpsimd.partition_broadcast(bc[:], recs[hh][:])
            bcs.append(bc)
        for hh in range(2):
            hr = hh * 64
            if last:
                nc.vector.tensor_mul(dsts[hh], avp[hh][0:64, :],
                                     bcs[hh][hr: hr + 64, :])
            else:
                nc.vector.tensor_mul(dsts[hh], dsts[hh], bcs[hh][hr: hr + 64, :])

    # ---- attention: flat 128-step pipeline, AV trailing scores by 2 ----
    pending = {}    # step -> ex tile
    block_av = {}   # block -> (av_h0, av_h1)
    for step in range(130):
        if step < 128:
            b, t = step // 16, step % 16
            ht, jw = b // 4, b % 4
            if t == 0:
                block_av[b] = tuple(
                    av_psum.tile([65, W], FP32, tag="av", name="avt")
                    for _ in range(2))
            sc = sc_psum.tile([128, 2 * W], FP32, tag="sc", name="sct")
            for hh in range(2):
                hr = hh * 64
                nc.tensor.matmul(
                    sc[:, hh * W: (hh + 1) * W],
                    lhsT=kt_sb[ht][hr: hr + 64, t * 128: t * 128 + 128],
                    rhs=qt_sb[ht][hr: hr + 64, jw * W: (jw + 1) * W],
                    start=True, stop=True)
            ex = exp_pool.tile([128, 2 * W], BF16, tag="exp", name="ex")
            nc.scalar.activation(ex[:], sc[:],
                                 mybir.ActivationFunctionType.Exp,
                                 bias=ebias_sb[:], scale=SCALE)
            pending[step] = ex
        for fn, a in work[step]:
            fn(*a)
        ps = step - 2
        if ps >= 0:
            ex = pending.pop(ps)
            b2, t2 = ps // 16, ps % 16
            ht2, jw2 = b2 // 4, b2 % 4
            for hh in range(2):
                h = ht2 * 2 + hh
                nc.tensor.matmul(
                    block_av[b2][hh][:],
                    lhsT=vaug_sb[t2][:, h * 65: h * 65 + 65],
                    rhs=ex[:, hh * W: (hh + 1) * W],
                    start=(t2 == 0), stop=(t2 == ST - 1))
            if t2 == ST - 1:
                finalize_pair(ht2, jw2, block_av.pop(b2), last=(b2 == 7))

    # ---- tail: junk matmuls keep HAM warm while the last finalize's
    # DVE/GpSimd chain runs, then the last window's output projection ----
    wup2 = pjv_psum.tile([128, 512], FP32, tag="pj", name="wup2")
    for i in range(12):
        nc.tensor.matmul(wup2[:], lhsT=wsrc[:, 0:128], rhs=wsrc[:],
                         start=(i == 0), stop=(i == 11))
    for u in range(8):
        emit_outproj_unit(3, u, use_act=(u % 2 == 1))


def _build_full(S=S_FULL):
    nc = bacc.Bacc("TRN2", target_bir_lowering=False, debug=False,
                   num_devices=N_CORES)
    xt = nc.dram_tensor("xt", [NW, 128, KT, 512], BF16, kind="ExternalInput")
    wq = nc.dram_tensor("wq", [128, KT * CW], BF16, kind="ExternalInput")
    wk = nc.dram_tensor("wk", [128, KT * CW], BF16, kind="ExternalInput")
    wv = nc.dram_tensor("wv", [128, KT * CW], BF16, kind="ExternalInput")
    wo = nc.dram_tensor("wo", [128, 2 * D], BF16, kind="ExternalInput")
    bias = nc.dram_tensor("bias", [2, 2, 128], FP32, kind="ExternalInput")
    out = nc.dram_tensor("out", [S, D], BF16, kind="ExternalOutput")
    ins = {k: t.ap() for k, t in
           dict(xt=xt, wq=wq, wk=wk, wv=wv, wo=wo, bias=bias).items()}
    with tile.TileContext(nc) as tc:
        with ExitStack() as ctx:
            build_mha_kernel(ctx, tc, out.ap(), ins, S=S)
    nc.compile()
    return nc


def make_core_inputs(X, Wq, bq, Wk, bk, Wv, bv, Wo, bo, S=S_FULL):
    """Build the 8 per-core input maps (host-side shard + transpose + cast)."""
    def pack_w(Wm, cs):   # [D, CW] slice -> SBUF layout [128, KT*CW]
        return np.ascontiguousarray(
            Wm[:, cs:cs + CW].astype(NP_BF16).reshape(KT, 128, CW)
            .transpose(1, 0, 2).reshape(128, KT * CW))

    in_maps = []
    xts = []
    for b in range(B):
        # X^T quarter-major: [q][p][k][c], 8KB contiguous per (q, p)
        xt = (X[b].T.astype(NP_BF16).reshape(KT, 128, NW, 512)
              .transpose(2, 1, 0, 3))
        xts.append(np.ascontiguousarray(xt))
    for c in range(N_CORES):
        b = c // (N_CORES // B)
        cs = (c % (N_CORES // B)) * CW
        m = {
            "xt": xts[b],
            "wq": pack_w(Wq, cs),
            "wk": pack_w(Wk, cs),
            "wv": pack_w(Wv, cs),
            "wo": np.ascontiguousarray(
                Wo[cs:cs + CW, :].astype(NP_BF16).reshape(2, 128, D)
                .transpose(1, 0, 2).reshape(128, 2 * D)),
            "bias": np.stack([bq[cs:cs + CW].reshape(2, 128),
                              bk[cs:cs + CW].reshape(2, 128)]).astype(np.float32),
        }
        in_maps.append(m)
    return in_maps


_NC_CACHE = {}


def _ensure_ntff_hook():
    """Register the axon NTFF profile hook if the image's antenv lacks it."""
    try:
        from antenv.axon_hooks import get_axon_ntff_profile_hook  # noqa: F401
        return  # already available
    except ImportError:
        pass
    try:
        import types
        import antenv
        from trn_agent_boot.trn_boot import _ntff_profile_via_ctypes
        hook = _ntff_profile_via_ctypes("/opt/axon/libaxon_pjrt.so")
        mod = types.ModuleType("antenv.axon_hooks")
        mod._hook = hook
        mod.get_axon_ntff_profile_hook = lambda: mod._hook
        mod.set_axon_ntff_profile_hook = lambda h: setattr(mod, "_hook", h)
        sys.modules["antenv.axon_hooks"] = mod
        antenv.axon_hooks = mod
    except Exception as e:  # profiling degrades, run still works
        print(f"ntff hook setup failed: {e}", file=sys.stderr)


def run_cores(in_maps, S=S_FULL, trace=False, trace_cores=None):
    if trace:
        _ensure_ntff_hook()
    if S not in _NC_CACHE:
        _NC_CACHE[S] = _build_full(S)
    nc = _NC_CACHE[S]
    return bass_utils.run_bass_kernel_spmd(
        nc, in_maps, core_ids=list(range(N_CORES)),
        trace=trace, trace_cores=trace_cores)


def kernel(X, Wq, bq, Wk, bk, Wv, bv, Wo, bo):
    X = np.asarray(X, dtype=np.float32)
    Wq, Wk, Wv, Wo = (np.asarray(w, dtype=np.float32) for w in (Wq, Wk, Wv, Wo))
    bq, bk, bv, bo = (np.asarray(v, dtype=np.float32) for v in (bq, bk, bv, bo))
    S = X.shape[1]
    in_maps = make_core_inputs(X, Wq, bq, Wk, bk, Wv, bv, Wo, bo, S=S)
    res = run_cores(in_maps, S=S)
    out = np.zeros((B, S, D), dtype=np.float32)
    for c in range(N_CORES):
        out[c // (N_CORES // B)] += np.asarray(res.results[c]["out"],
                                               dtype=np.float32)
    out += bo + bv @ Wo   # V bias commutes through softmax
    return out
